# revision 7
# baseline (speedup 1.0000x reference)
"""Trainium2 Bass kernel for an 8-head GLU multi-head self-attention block.

Shapes (hardcoded from the problem spec):
  x [4, 2048, 1024], mask [4, 2048] (int32),
  W_q/W_k [1024, 2048], W_v [1024, 4096], W_o [2048, 2048],
  b_q/b_k [2048], b_v [4096], b_o [2048]  ->  out [4, 2048, 1024] f32.

Sharding: 8 cores = 4 batches x 2 query-halves. Each core computes K/V
projections for its full batch (duplicated within the pair - keeps the
program collective-free and fully static for SPMD), Q projection +
attention + output projection + GLUs for its 1024-query half, all 8 heads.

All matmuls run as float32r (full PE rate at N>=512, ~1e-4 rounding).
Layouts keep the contraction dim on SBUF partitions throughout:
  QT/KT [dk, q], scores transposed [k, q] (exp'd on ACT), V natural [k, dv]
so attention needs no on-chip transposes. Softmax denominator comes from a
mask-column matmul over the exp'd scores; sigmoid is computed via tanh to
stay inside the single ACT table set (exp/tanh/copy).
"""

import sys
import numpy as np

for _p in ("/opt/trn_rl_repo", "/root/.axon_site/_ro/trn_rl_repo"):
    if _p not in sys.path:
        sys.path.insert(0, _p)

import concourse.bass as bass
import concourse.mybir as mybir
import concourse.tile as tile
from concourse import bacc
from concourse.bass_utils import run_bass_kernel_spmd

F32 = mybir.dt.float32
F32R = mybir.dt.float32r
AL = mybir.AluOpType
AF = mybir.ActivationFunctionType

N_CORES = 8
S = 2048          # sequence length
D = 1024          # d_model
H = 8             # heads
DK = 256          # per-head q/k dim
DV = 512          # per-head v dim (GLU-doubled)
DO = 2048         # output-projection dim (GLU-doubled)
QH = S // 2       # queries per core


def _bcast_ap(vec_ap, parts, offset, n):
    """AP reading vec[offset:offset+n] broadcast across `parts` partitions."""
    return bass.AP(tensor=vec_ap.tensor, offset=offset, ap=[[0, parts], [1, n]])


def _build():
    nc = bacc.Bacc("TRN2", target_bir_lowering=False, debug=False,
                   num_devices=N_CORES)

    xT = nc.dram_tensor("xT", [D, S], F32, kind="ExternalInput").ap()
    xTq = nc.dram_tensor("xTq", [D, QH], F32, kind="ExternalInput").ap()
    wq = nc.dram_tensor("wq", [D, H * DK], F32, kind="ExternalInput").ap()
    wk = nc.dram_tensor("wk", [D, H * DK], F32, kind="ExternalInput").ap()
    wv = nc.dram_tensor("wv", [D, H * DV], F32, kind="ExternalInput").ap()
    wo = nc.dram_tensor("wo", [H * DK, DO], F32, kind="ExternalInput").ap()
    bq = nc.dram_tensor("bq", [H * DK], F32, kind="ExternalInput").ap()
    bk = nc.dram_tensor("bk", [H * DK], F32, kind="ExternalInput").ap()
    bv = nc.dram_tensor("bv", [H * DV], F32, kind="ExternalInput").ap()
    bo = nc.dram_tensor("bo", [DO], F32, kind="ExternalInput").ap()
    maskf = nc.dram_tensor("maskf", [S], F32, kind="ExternalInput").ap()
    out = nc.dram_tensor("out", [QH, D], F32, kind="ExternalOutput").ap()

    # DRAM spill for projections (written pass 1, streamed back pass 2/C).
    # Per-head tensors so pass-2 prefetch of head h doesn't wait on head h+1
    # writes (dependency tracking is per-tensor).
    QT_ds = [nc.dram_tensor(f"QT_d{h}", [DK, QH], F32).ap().bitcast(F32R) for h in range(H)]
    KT_ds = [nc.dram_tensor(f"KT_d{h}", [DK, S], F32).ap().bitcast(F32R) for h in range(H)]
    V_ds = [nc.dram_tensor(f"V_d{h}", [S, DV], F32).ap().bitcast(F32R) for h in range(H)]
    G_ds = [nc.dram_tensor(f"G_d{h}", [DK, QH], F32).ap().bitcast(F32R) for h in range(H)]

    with tile.TileContext(nc) as tc:
        with tc.tile_pool(name="consts", bufs=1) as consts:
            # Per-c-tile bias columns (f32: used as tensor_scalar operands).
            bq_cols = consts.tile([128, H * DK // 128], F32)
            bk_cols = consts.tile([128, H * DK // 128], F32)
            for ct in range(H * DK // 128):
                nc.sync.dma_start(out=bq_cols[:, ct:ct + 1],
                                  in_=bq[ct * 128:(ct + 1) * 128].rearrange("(p o) -> p o", o=1))
                nc.sync.dma_start(out=bk_cols[:, ct:ct + 1],
                                  in_=bk[ct * 128:(ct + 1) * 128].rearrange("(p o) -> p o", o=1))
            # Mask columns per k-tile: f32 (scalar operand), x0.5 variant, f32r (matmul lhsT).
            mcol = consts.tile([128, S // 128], F32)
            for kt in range(S // 128):
                nc.sync.dma_start(out=mcol[:, kt:kt + 1],
                                  in_=maskf[kt * 128:(kt + 1) * 128].rearrange("(p o) -> p o", o=1))
            mhalf = consts.tile([128, S // 128], F32)
            nc.vector.tensor_scalar_mul(mhalf, mcol, 0.5)
            mcol_r = consts.tile([128, S // 128], F32R)
            nc.vector.tensor_copy(mcol_r, mcol)
            # Row of ones (bc matmul lhsT, K=1).
            ones_f = consts.tile([1, 128], F32)
            nc.vector.memset(ones_f, 1.0)
            ones1 = consts.tile([1, 128], F32R)
            nc.vector.tensor_copy(ones1, ones_f)
            ones_c = consts.tile([128, 1], F32)
            nc.vector.memset(ones_c, 1.0)
            ones128 = consts.tile([128, 1], F32R)
            nc.vector.tensor_copy(ones128, ones_c)
            # b_o broadcast [128, DO]; plus 0.5*b_o[:D] for the final GLU.
            bo_bc = consts.tile([128, DO], F32)
            nc.gpsimd.dma_start(out=bo_bc, in_=_bcast_ap(bo, 128, 0, DO))
            bo1h = consts.tile([128, D], F32)
            nc.vector.tensor_scalar_mul(bo1h, bo_bc[:, 0:D], 0.5)

            # ---------------- Pass 1: QKV projections -> DRAM ----------------
            with tc.tile_pool(name="p1", bufs=2) as p1, \
                 tc.tile_pool(name="ps1", bufs=2, space="PSUM") as ps1:
                xT_sb = []
                for d in range(D // 128):
                    t = p1.tile([128, S], F32R, tag="xT_sb", bufs=8)
                    nc.sync.dma_start(out=t, in_=xT[d * 128:(d + 1) * 128, :].bitcast(F32R))
                    xT_sb.append(t)
                xTq_sb = []
                for d in range(D // 128):
                    t = p1.tile([128, QH], F32R, tag="xTq_sb", bufs=8)
                    nc.sync.dma_start(out=t, in_=xTq[d * 128:(d + 1) * 128, :].bitcast(F32R))
                    xTq_sb.append(t)

                for h in range(H):
                    # K^T and Q^T: [dk, seq] via lhsT=W-block [d,128c], rhs=xT [d, seq-chunk]
                    for ct in range(2):
                        c0 = h * DK + ct * 128
                        wkb = []
                        wqb = []
                        for d in range(D // 128):
                            tk = p1.tile([128, 128], F32R, tag="wkb", bufs=24)
                            nc.sync.dma_start(out=tk, in_=wk[d * 128:(d + 1) * 128, c0:c0 + 128].bitcast(F32R))
                            wkb.append(tk)
                            tq = p1.tile([128, 128], F32R, tag="wqb", bufs=24)
                            nc.sync.dma_start(out=tq, in_=wq[d * 128:(d + 1) * 128, c0:c0 + 128].bitcast(F32R))
                            wqb.append(tq)
                        for kc in range(S // 512):
                            ps = ps1.tile([128, 512], F32, tag="ps1")
                            for d in range(D // 128):
                                nc.tensor.matmul(ps, wkb[d], xT_sb[d][:, kc * 512:(kc + 1) * 512],
                                                 start=(d == 0), stop=(d == D // 128 - 1))
                            ev = p1.tile([128, 512], F32R, tag="ev", bufs=8)
                            nc.vector.tensor_scalar(ev, ps, bk_cols[:, h * 2 + ct:h * 2 + ct + 1],
                                                    None, op0=AL.add)
                            nc.sync.dma_start(out=KT_ds[h][ct * 128:ct * 128 + 128, kc * 512:(kc + 1) * 512], in_=ev)
                        for qc in range(QH // 512):
                            ps = ps1.tile([128, 512], F32, tag="ps1")
                            for d in range(D // 128):
                                nc.tensor.matmul(ps, wqb[d], xTq_sb[d][:, qc * 512:(qc + 1) * 512],
                                                 start=(d == 0), stop=(d == D // 128 - 1))
                            ev = p1.tile([128, 512], F32R, tag="ev", bufs=8)
                            nc.vector.tensor_scalar(ev, ps, bq_cols[:, h * 2 + ct:h * 2 + ct + 1],
                                                    0.0625, op0=AL.add, op1=AL.mult)
                            nc.sync.dma_start(out=QT_ds[h][ct * 128:ct * 128 + 128, qc * 512:(qc + 1) * 512], in_=ev)
                    # V natural [k, dv]: lhsT=xT-block [d, 128k], rhs=wv [d, 512dv]
                    v0 = h * DV
                    wvb = []
                    for d in range(D // 128):
                        t = p1.tile([128, DV], F32R, tag="wvb", bufs=16)
                        nc.sync.dma_start(out=t, in_=wv[d * 128:(d + 1) * 128, v0:v0 + DV].bitcast(F32R))
                        wvb.append(t)
                    bva = p1.tile([128, 256], F32, tag="bva", bufs=4)
                    nc.gpsimd.dma_start(out=bva, in_=_bcast_ap(bv, 128, v0, 256))
                    bvah = p1.tile([128, 256], F32, tag="bvah", bufs=4)
                    nc.vector.tensor_scalar_mul(bvah, bva, 0.5)
                    bvg = p1.tile([128, 256], F32, tag="bvg", bufs=4)
                    nc.gpsimd.dma_start(out=bvg, in_=_bcast_ap(bv, 128, v0 + 256, 256))
                    for kt in range(S // 128):
                        ps = ps1.tile([128, 512], F32, tag="ps1")
                        for d in range(D // 128):
                            nc.tensor.matmul(ps, xT_sb[d][:, kt * 128:(kt + 1) * 128], wvb[d],
                                             start=(d == 0), stop=(d == D // 128 - 1))
                        ev = p1.tile([128, 512], F32R, tag="ev", bufs=8)
                        # a-half gets the extra 0.5 of the GLU-sigmoid identity folded in
                        nc.vector.scalar_tensor_tensor(ev[:, 0:256], ps[:, 0:256],
                                                       mhalf[:, kt:kt + 1], bvah,
                                                       op0=AL.mult, op1=AL.add)
                        nc.vector.scalar_tensor_tensor(ev[:, 256:512], ps[:, 256:512],
                                                       mcol[:, kt:kt + 1], bvg,
                                                       op0=AL.mult, op1=AL.add)
                        nc.sync.dma_start(out=V_ds[h][kt * 128:(kt + 1) * 128, :], in_=ev)

            # ---------------- Pass 2: attention per head ----------------
            with tc.tile_pool(name="p2", bufs=2) as p2, \
                 tc.tile_pool(name="ps_st", bufs=2, space="PSUM") as ps_st, \
                 tc.tile_pool(name="ps_ot", bufs=4, space="PSUM") as ps_ot, \
                 tc.tile_pool(name="ps_dn", bufs=1, space="PSUM") as ps_dn, \
                 tc.tile_pool(name="ps_bc", bufs=1, space="PSUM") as ps_bc:
                for h in range(H):
                    QT_h = []
                    KT_h = []
                    for ct in range(2):
                        c0 = h * DK + ct * 128
                        tq = p2.tile([128, QH], F32R, tag="qt", bufs=4)
                        nc.sync.dma_start(out=tq, in_=QT_ds[h][ct * 128:ct * 128 + 128, :])
                        QT_h.append(tq)
                        tk = p2.tile([128, S], F32R, tag="kt", bufs=4)
                        nc.sync.dma_start(out=tk, in_=KT_ds[h][ct * 128:ct * 128 + 128, :])
                        KT_h.append(tk)
                    V_h = []
                    for kt in range(S // 128):
                        tv = p2.tile([128, DV], F32R, tag="vt", bufs=24)
                        nc.sync.dma_start(out=tv, in_=V_ds[h][kt * 128:(kt + 1) * 128, :])
                        V_h.append(tv)
                    for qc in range(QH // 512):
                        q0 = qc * 512
                        ET = []
                        for kt in range(S // 128):
                            st = ps_st.tile([128, 512], F32, tag="st")
                            nc.tensor.matmul(st, KT_h[0][:, kt * 128:(kt + 1) * 128],
                                             QT_h[0][:, q0:q0 + 512], start=True, stop=False)
                            nc.tensor.matmul(st, KT_h[1][:, kt * 128:(kt + 1) * 128],
                                             QT_h[1][:, q0:q0 + 512], start=False, stop=True)
                            e = p2.tile([128, 512], F32R, tag="et", bufs=28)
                            nc.scalar.activation(e, st, AF.Exp)
                            ET.append(e)
                        ots = [ps_ot.tile([128, 512], F32, tag="ot", name=f"ot{_i}") for _i in range(4)]
                        den = ps_dn.tile([1, 512], F32, tag="den")
                        for kt in range(S // 128):
                            first, last = kt == 0, kt == S // 128 - 1
                            for dvt in range(4):
                                nc.tensor.matmul(ots[dvt], V_h[kt][:, dvt * 128:(dvt + 1) * 128],
                                                 ET[kt], start=first, stop=last)
                            nc.tensor.matmul(den, mcol_r[:, kt:kt + 1], ET[kt],
                                             start=first, stop=last)
                        rc = p2.tile([1, 512], F32R, tag="rc", bufs=2)
                        with nc.allow_low_precision(reason="f32r recip feeds f32r matmul"):
                            nc.vector.reciprocal(rc, den)
                        bcp = ps_bc.tile([128, 512], F32, tag="bcp")
                        nc.tensor.matmul(bcp, ones1, rc, start=True, stop=True)
                        bc = p2.tile([128, 512], F32, tag="bc", bufs=2)
                        nc.scalar.activation(bc, bcp, AF.Copy)
                        for c2 in range(2):
                            an = p2.tile([128, 512], F32, tag="an", bufs=2)
                            nc.vector.tensor_tensor(an, ots[c2], bc, AL.mult)
                            gn = p2.tile([128, 512], F32, tag="gn", bufs=2)
                            nc.vector.tensor_tensor(gn, ots[2 + c2], bc, AL.mult)
                            tg = p2.tile([128, 512], F32, tag="tg", bufs=2)
                            nc.scalar.activation(tg, gn, AF.Tanh, scale=0.5)
                            go = p2.tile([128, 512], F32R, tag="go", bufs=4)
                            nc.vector.scalar_tensor_tensor(go, tg, 1.0, an,
                                                           op0=AL.add, op1=AL.mult)
                            nc.sync.dma_start(out=G_ds[h][c2 * 128:(c2 + 1) * 128,
                                                          q0:q0 + 512], in_=go)

            # ---------------- Phase C: output projection + GLU ----------------
            with tc.tile_pool(name="pc", bufs=2) as pc, \
                 tc.tile_pool(name="ps_y", bufs=4, space="PSUM") as ps_y:
                G_sb = []
                for ct in range(H * DK // 128):
                    t = pc.tile([128, QH], F32R, tag="g_sb", bufs=16)
                    nc.sync.dma_start(out=t, in_=G_ds[ct // 2][(ct % 2) * 128:(ct % 2) * 128 + 128, :])
                    G_sb.append(t)
                for pair in range(2):
                    o1c = pair * 512
                    o2c = D + pair * 512
                    wo1 = []
                    wo2 = []
                    for ct in range(H * DK // 128):
                        t1 = pc.tile([128, 512], F32R, tag="wo1", bufs=16)
                        nc.sync.dma_start(out=t1, in_=wo[ct * 128:(ct + 1) * 128,
                                                        o1c:o1c + 512].bitcast(F32R))
                        wo1.append(t1)
                        t2 = pc.tile([128, 512], F32R, tag="wo2", bufs=16)
                        nc.sync.dma_start(out=t2, in_=wo[ct * 128:(ct + 1) * 128,
                                                        o2c:o2c + 512].bitcast(F32R))
                        wo2.append(t2)
                    for qs in range(QH // 128):
                        y1 = ps_y.tile([128, 512], F32, tag="y")
                        y2 = ps_y.tile([128, 512], F32, tag="y")
                        for ct in range(H * DK // 128):
                            nc.tensor.matmul(y1, G_sb[ct][:, qs * 128:(qs + 1) * 128], wo1[ct],
                                             start=(ct == 0), stop=(ct == H * DK // 128 - 1))
                        for ct in range(H * DK // 128):
                            nc.tensor.matmul(y2, G_sb[ct][:, qs * 128:(qs + 1) * 128], wo2[ct],
                                             start=(ct == 0), stop=(ct == H * DK // 128 - 1))
                        y2b = pc.tile([128, 512], F32, tag="y2b", bufs=3)
                        nc.vector.tensor_tensor(y2b, y2, bo_bc[:, o2c:o2c + 512], AL.add)
                        t2 = pc.tile([128, 512], F32, tag="t2", bufs=3)
                        nc.scalar.activation(t2, y2b, AF.Tanh, scale=0.5)
                        y1b = pc.tile([128, 512], F32, tag="y1b", bufs=3)
                        nc.vector.scalar_tensor_tensor(y1b, y1, 0.5, bo1h[:, o1c:o1c + 512],
                                                       op0=AL.mult, op1=AL.add)
                        oc = pc.tile([128, 512], F32, tag="oc", bufs=4)
                        nc.vector.scalar_tensor_tensor(oc, t2, 1.0, y1b,
                                                       op0=AL.add, op1=AL.mult)
                        nc.sync.dma_start(out=out[qs * 128:(qs + 1) * 128, o1c:o1c + 512], in_=oc)

    nc.compile()
    return nc


_NC = None


def kernel(**inputs):
    global _NC
    x = np.ascontiguousarray(np.asarray(inputs["x"], dtype=np.float32))
    mask = np.asarray(inputs["mask"])
    W_q = np.ascontiguousarray(np.asarray(inputs["W_q"], dtype=np.float32))
    W_k = np.ascontiguousarray(np.asarray(inputs["W_k"], dtype=np.float32))
    W_v = np.ascontiguousarray(np.asarray(inputs["W_v"], dtype=np.float32))
    W_o = np.ascontiguousarray(np.asarray(inputs["W_o"], dtype=np.float32))
    b_q = np.ascontiguousarray(np.asarray(inputs["b_q"], dtype=np.float32))
    b_k = np.ascontiguousarray(np.asarray(inputs["b_k"], dtype=np.float32))
    b_v = np.ascontiguousarray(np.asarray(inputs["b_v"], dtype=np.float32))
    b_o = np.ascontiguousarray(np.asarray(inputs["b_o"], dtype=np.float32))

    if _NC is None:
        _NC = _build()

    B = x.shape[0]
    in_maps = []
    for core in range(N_CORES):
        b, g = core // 2, core % 2
        xT = np.ascontiguousarray(x[b].T)
        in_maps.append({
            "xT": xT,
            "xTq": np.ascontiguousarray(xT[:, g * QH:(g + 1) * QH]),
            "wq": W_q, "wk": W_k, "wv": W_v, "wo": W_o,
            "bq": b_q, "bk": b_k, "bv": b_v, "bo": b_o,
            "maskf": np.ascontiguousarray(mask[b].astype(np.float32)),
        })

    res = run_bass_kernel_spmd(_NC, in_maps, core_ids=list(range(N_CORES)))
    out = np.empty((B, S, D), dtype=np.float32)
    for core in range(N_CORES):
        b, g = core // 2, core % 2
        out[b, g * QH:(g + 1) * QH, :] = res.results[core]["out"]
    return out


# revision 8
# speedup vs baseline: 1.0340x; 1.0340x over previous
"""Trainium2 Bass kernel for an 8-head GLU multi-head self-attention block.

Shapes (hardcoded from the problem spec):
  x [4, 2048, 1024], mask [4, 2048] (int32),
  W_q/W_k [1024, 2048], W_v [1024, 4096], W_o [2048, 2048],
  b_q/b_k [2048], b_v [4096], b_o [2048]  ->  out [4, 2048, 1024] f32.

Sharding: 8 cores = 4 batches x 2 query-halves. Each core computes K/V
projections for its full batch (duplicated within the pair - keeps the
program collective-free and fully static for SPMD), Q projection +
attention + output projection + GLUs for its 1024-query half, all 8 heads.

All matmuls run as float32r (full PE rate at N>=512, ~1e-4 rounding).
Layouts keep the contraction dim on SBUF partitions throughout:
  QT/KT [dk, q], scores transposed [k, q] (exp'd on ACT), V natural [k, dv]
so attention needs no on-chip transposes. Softmax denominator comes from a
mask-column matmul over the exp'd scores; sigmoid is computed via tanh to
stay inside the single ACT table set (exp/tanh/copy).
"""

import sys
import numpy as np

for _p in ("/opt/trn_rl_repo", "/root/.axon_site/_ro/trn_rl_repo"):
    if _p not in sys.path:
        sys.path.insert(0, _p)

import concourse.bass as bass
import concourse.mybir as mybir
import concourse.tile as tile
from concourse import bacc
from concourse.bass_utils import run_bass_kernel_spmd

F32 = mybir.dt.float32
F32R = mybir.dt.float32r
AL = mybir.AluOpType
AF = mybir.ActivationFunctionType

N_CORES = 8
S = 2048          # sequence length
D = 1024          # d_model
H = 8             # heads
DK = 256          # per-head q/k dim
DV = 512          # per-head v dim (GLU-doubled)
DO = 2048         # output-projection dim (GLU-doubled)
QH = S // 2       # queries per core


def _bcast_ap(vec_ap, parts, offset, n):
    """AP reading vec[offset:offset+n] broadcast across `parts` partitions."""
    return bass.AP(tensor=vec_ap.tensor, offset=offset, ap=[[0, parts], [1, n]])


def _build():
    nc = bacc.Bacc("TRN2", target_bir_lowering=False, debug=False,
                   num_devices=N_CORES)

    xT = nc.dram_tensor("xT", [D, S], F32, kind="ExternalInput").ap()
    xTq = nc.dram_tensor("xTq", [D, QH], F32, kind="ExternalInput").ap()
    wq = nc.dram_tensor("wq", [D, H * DK], F32, kind="ExternalInput").ap()
    wk = nc.dram_tensor("wk", [D, H * DK], F32, kind="ExternalInput").ap()
    wv = nc.dram_tensor("wv", [D, H * DV], F32, kind="ExternalInput").ap()
    wo = nc.dram_tensor("wo", [H * DK, DO], F32, kind="ExternalInput").ap()
    bq = nc.dram_tensor("bq", [H * DK], F32, kind="ExternalInput").ap()
    bk = nc.dram_tensor("bk", [H * DK], F32, kind="ExternalInput").ap()
    bv = nc.dram_tensor("bv", [H * DV], F32, kind="ExternalInput").ap()
    bo = nc.dram_tensor("bo", [DO], F32, kind="ExternalInput").ap()
    maskf = nc.dram_tensor("maskf", [S], F32, kind="ExternalInput").ap()
    out = nc.dram_tensor("out", [QH, D], F32, kind="ExternalOutput").ap()

    # DRAM spill for projections (written pass 1, streamed back pass 2/C).
    # Per-head tensors so pass-2 prefetch of head h doesn't wait on head h+1
    # writes (dependency tracking is per-tensor).
    QT_ds = [nc.dram_tensor(f"QT_d{h}", [DK, QH], F32).ap().bitcast(F32R) for h in range(H)]
    KT_ds = [nc.dram_tensor(f"KT_d{h}", [DK, S], F32).ap().bitcast(F32R) for h in range(H)]
    V_ds = [nc.dram_tensor(f"V_d{h}", [S, DV], F32).ap().bitcast(F32R) for h in range(H)]
    G_ds = [nc.dram_tensor(f"G_d{h}", [DK, QH], F32).ap().bitcast(F32R) for h in range(H)]

    with tile.TileContext(nc) as tc:
        with tc.tile_pool(name="consts", bufs=1) as consts:
            # Per-c-tile bias columns (f32: used as tensor_scalar operands).
            bq_cols = consts.tile([128, H * DK // 128], F32)
            bk_cols = consts.tile([128, H * DK // 128], F32)
            for ct in range(H * DK // 128):
                nc.sync.dma_start(out=bq_cols[:, ct:ct + 1],
                                  in_=bq[ct * 128:(ct + 1) * 128].rearrange("(p o) -> p o", o=1))
                nc.sync.dma_start(out=bk_cols[:, ct:ct + 1],
                                  in_=bk[ct * 128:(ct + 1) * 128].rearrange("(p o) -> p o", o=1))
            # Mask columns per k-tile: f32 (scalar operand), x0.5 variant, f32r (matmul lhsT).
            mcol = consts.tile([128, S // 128], F32)
            for kt in range(S // 128):
                nc.sync.dma_start(out=mcol[:, kt:kt + 1],
                                  in_=maskf[kt * 128:(kt + 1) * 128].rearrange("(p o) -> p o", o=1))
            mhalf = consts.tile([128, S // 128], F32)
            nc.vector.tensor_scalar_mul(mhalf, mcol, 0.5)
            mcol_r = consts.tile([128, S // 128], F32R)
            nc.vector.tensor_copy(mcol_r, mcol)
            # Row of ones (bc matmul lhsT, K=1).
            ones_f = consts.tile([1, 128], F32)
            nc.vector.memset(ones_f, 1.0)
            ones1 = consts.tile([1, 128], F32R)
            nc.vector.tensor_copy(ones1, ones_f)
            ones_c = consts.tile([128, 1], F32)
            nc.vector.memset(ones_c, 1.0)
            ones128 = consts.tile([128, 1], F32R)
            nc.vector.tensor_copy(ones128, ones_c)
            # b_o broadcast [128, DO]; plus 0.5*b_o[:D] for the final GLU.
            bo_bc = consts.tile([128, DO], F32)
            nc.gpsimd.dma_start(out=bo_bc, in_=_bcast_ap(bo, 128, 0, DO))
            bo1h = consts.tile([128, D], F32)
            nc.vector.tensor_scalar_mul(bo1h, bo_bc[:, 0:D], 0.5)

            # ---------------- Pass 1: QKV projections -> DRAM ----------------
            with tc.tile_pool(name="p1", bufs=2) as p1, \
                 tc.tile_pool(name="ps1", bufs=2, space="PSUM") as ps1:
                xT_sb = []
                for d in range(D // 128):
                    t = p1.tile([128, S], F32R, tag="xT_sb", bufs=8)
                    nc.sync.dma_start(out=t, in_=xT[d * 128:(d + 1) * 128, :].bitcast(F32R))
                    xT_sb.append(t)
                xTq_sb = []
                for d in range(D // 128):
                    t = p1.tile([128, QH], F32R, tag="xTq_sb", bufs=8)
                    nc.sync.dma_start(out=t, in_=xTq[d * 128:(d + 1) * 128, :].bitcast(F32R))
                    xTq_sb.append(t)

                for h in range(H):
                    # K^T and Q^T: [dk, seq] via lhsT=W-block [d,128c], rhs=xT [d, seq-chunk]
                    for ct in range(2):
                        c0 = h * DK + ct * 128
                        wkb = []
                        wqb = []
                        for d in range(D // 128):
                            tk = p1.tile([128, 128], F32R, tag="wkb", bufs=24)
                            nc.sync.dma_start(out=tk, in_=wk[d * 128:(d + 1) * 128, c0:c0 + 128].bitcast(F32R))
                            wkb.append(tk)
                            tq = p1.tile([128, 128], F32R, tag="wqb", bufs=24)
                            nc.sync.dma_start(out=tq, in_=wq[d * 128:(d + 1) * 128, c0:c0 + 128].bitcast(F32R))
                            wqb.append(tq)
                        for kc in range(S // 512):
                            ps = ps1.tile([128, 512], F32, tag="ps1")
                            for d in range(D // 128):
                                nc.tensor.matmul(ps, wkb[d], xT_sb[d][:, kc * 512:(kc + 1) * 512],
                                                 start=(d == 0), stop=(d == D // 128 - 1))
                            ev = p1.tile([128, 512], F32R, tag="ev", bufs=8)
                            nc.vector.tensor_scalar(ev, ps, bk_cols[:, h * 2 + ct:h * 2 + ct + 1],
                                                    None, op0=AL.add)
                            nc.sync.dma_start(out=KT_ds[h][ct * 128:ct * 128 + 128, kc * 512:(kc + 1) * 512], in_=ev)
                        for qc in range(QH // 512):
                            ps = ps1.tile([128, 512], F32, tag="ps1")
                            for d in range(D // 128):
                                nc.tensor.matmul(ps, wqb[d], xTq_sb[d][:, qc * 512:(qc + 1) * 512],
                                                 start=(d == 0), stop=(d == D // 128 - 1))
                            ev = p1.tile([128, 512], F32R, tag="ev", bufs=8)
                            nc.vector.tensor_scalar(ev, ps, bq_cols[:, h * 2 + ct:h * 2 + ct + 1],
                                                    0.0625, op0=AL.add, op1=AL.mult)
                            nc.sync.dma_start(out=QT_ds[h][ct * 128:ct * 128 + 128, qc * 512:(qc + 1) * 512], in_=ev)
                    # V natural [k, dv]: lhsT=xT-block [d, 128k], rhs=wv [d, 512dv]
                    v0 = h * DV
                    wvb = []
                    for d in range(D // 128):
                        t = p1.tile([128, DV], F32R, tag="wvb", bufs=16)
                        nc.sync.dma_start(out=t, in_=wv[d * 128:(d + 1) * 128, v0:v0 + DV].bitcast(F32R))
                        wvb.append(t)
                    bva = p1.tile([128, 256], F32, tag="bva", bufs=4)
                    nc.gpsimd.dma_start(out=bva, in_=_bcast_ap(bv, 128, v0, 256))
                    bvah = p1.tile([128, 256], F32, tag="bvah", bufs=4)
                    nc.vector.tensor_scalar_mul(bvah, bva, 0.5)
                    bvg = p1.tile([128, 256], F32, tag="bvg", bufs=4)
                    nc.gpsimd.dma_start(out=bvg, in_=_bcast_ap(bv, 128, v0 + 256, 256))
                    for kt in range(S // 128):
                        ps = ps1.tile([128, 512], F32, tag="ps1")
                        for d in range(D // 128):
                            nc.tensor.matmul(ps, xT_sb[d][:, kt * 128:(kt + 1) * 128], wvb[d],
                                             start=(d == 0), stop=(d == D // 128 - 1))
                        ev = p1.tile([128, 512], F32R, tag="ev", bufs=8)
                        # a-half gets the extra 0.5 of the GLU-sigmoid identity folded in
                        nc.vector.scalar_tensor_tensor(ev[:, 0:256], ps[:, 0:256],
                                                       mhalf[:, kt:kt + 1], bvah,
                                                       op0=AL.mult, op1=AL.add)
                        nc.vector.scalar_tensor_tensor(ev[:, 256:512], ps[:, 256:512],
                                                       mcol[:, kt:kt + 1], bvg,
                                                       op0=AL.mult, op1=AL.add)
                        nc.sync.dma_start(out=V_ds[h][kt * 128:(kt + 1) * 128, :], in_=ev)

            # ---------------- Pass 2: attention per head ----------------
            with tc.tile_pool(name="p2", bufs=2) as p2, \
                 tc.tile_pool(name="ps_st", bufs=3, space="PSUM") as ps_st, \
                 tc.tile_pool(name="ps_ot", bufs=4, space="PSUM") as ps_ot, \
                 tc.tile_pool(name="ps_dn", bufs=1, space="PSUM") as ps_dn:
                for h in range(H):
                    QT_h = []
                    KT_h = []
                    for ct in range(2):
                        c0 = h * DK + ct * 128
                        tq = p2.tile([128, QH], F32R, tag="qt", bufs=4)
                        nc.sync.dma_start(out=tq, in_=QT_ds[h][ct * 128:ct * 128 + 128, :])
                        QT_h.append(tq)
                        tk = p2.tile([128, S], F32R, tag="kt", bufs=4)
                        nc.sync.dma_start(out=tk, in_=KT_ds[h][ct * 128:ct * 128 + 128, :])
                        KT_h.append(tk)
                    V_h = []
                    for kt in range(S // 128):
                        tv = p2.tile([128, DV], F32R, tag="vt", bufs=24)
                        nc.sync.dma_start(out=tv, in_=V_ds[h][kt * 128:(kt + 1) * 128, :])
                        V_h.append(tv)
                    for qc in range(QH // 512):
                        q0 = qc * 512
                        ET = []
                        for kt in range(S // 128):
                            st = ps_st.tile([128, 512], F32, tag="st")
                            nc.tensor.matmul(st, KT_h[0][:, kt * 128:(kt + 1) * 128],
                                             QT_h[0][:, q0:q0 + 512], start=True, stop=False)
                            nc.tensor.matmul(st, KT_h[1][:, kt * 128:(kt + 1) * 128],
                                             QT_h[1][:, q0:q0 + 512], start=False, stop=True)
                            e = p2.tile([128, 512], F32R, tag="et", bufs=28)
                            nc.scalar.activation(e, st, AF.Exp)
                            ET.append(e)
                        ots = [ps_ot.tile([128, 512], F32, tag="ot", name=f"ot{_i}") for _i in range(4)]
                        den = ps_dn.tile([1, 512], F32, tag="den")
                        for kt in range(S // 128):
                            first, last = kt == 0, kt == S // 128 - 1
                            for dvt in range(4):
                                nc.tensor.matmul(ots[dvt], V_h[kt][:, dvt * 128:(dvt + 1) * 128],
                                                 ET[kt], start=first, stop=last)
                            nc.tensor.matmul(den, mcol_r[:, kt:kt + 1], ET[kt],
                                             start=first, stop=last)
                        dsb = p2.tile([1, 512], F32R, tag="dsb", bufs=2)
                        nc.scalar.activation(dsb, den, AF.Copy)
                        bcp = ps_dn.tile([128, 512], F32, tag="den")
                        nc.tensor.matmul(bcp, ones1, dsb, start=True, stop=True)
                        bc = p2.tile([128, 512], F32, tag="bc", bufs=2)
                        with nc.allow_low_precision(reason="recip of broadcast denominator"):
                            nc.vector.reciprocal(bc, bcp)
                        for c2 in range(2):
                            an = p2.tile([128, 512], F32, tag="an", bufs=2)
                            nc.vector.tensor_tensor(an, ots[c2], bc, AL.mult)
                            gn = p2.tile([128, 512], F32, tag="gn", bufs=2)
                            nc.vector.tensor_tensor(gn, ots[2 + c2], bc, AL.mult)
                            tg = p2.tile([128, 512], F32, tag="tg", bufs=2)
                            nc.scalar.activation(tg, gn, AF.Tanh, scale=0.5)
                            go = p2.tile([128, 512], F32R, tag="go", bufs=4)
                            nc.vector.scalar_tensor_tensor(go, tg, 1.0, an,
                                                           op0=AL.add, op1=AL.mult)
                            nc.sync.dma_start(out=G_ds[h][c2 * 128:(c2 + 1) * 128,
                                                          q0:q0 + 512], in_=go)

            # ---------------- Phase C: output projection + GLU ----------------
            with tc.tile_pool(name="pc", bufs=2) as pc, \
                 tc.tile_pool(name="ps_y", bufs=4, space="PSUM") as ps_y:
                G_sb = []
                for ct in range(H * DK // 128):
                    t = pc.tile([128, QH], F32R, tag="g_sb", bufs=16)
                    nc.sync.dma_start(out=t, in_=G_ds[ct // 2][(ct % 2) * 128:(ct % 2) * 128 + 128, :])
                    G_sb.append(t)
                for pair in range(2):
                    o1c = pair * 512
                    o2c = D + pair * 512
                    wo1 = []
                    wo2 = []
                    for ct in range(H * DK // 128):
                        t1 = pc.tile([128, 512], F32R, tag="wo1", bufs=16)
                        nc.sync.dma_start(out=t1, in_=wo[ct * 128:(ct + 1) * 128,
                                                        o1c:o1c + 512].bitcast(F32R))
                        wo1.append(t1)
                        t2 = pc.tile([128, 512], F32R, tag="wo2", bufs=16)
                        nc.sync.dma_start(out=t2, in_=wo[ct * 128:(ct + 1) * 128,
                                                        o2c:o2c + 512].bitcast(F32R))
                        wo2.append(t2)
                    for qs in range(QH // 128):
                        y1 = ps_y.tile([128, 512], F32, tag="y")
                        y2 = ps_y.tile([128, 512], F32, tag="y")
                        for ct in range(H * DK // 128):
                            nc.tensor.matmul(y1, G_sb[ct][:, qs * 128:(qs + 1) * 128], wo1[ct],
                                             start=(ct == 0), stop=(ct == H * DK // 128 - 1))
                        for ct in range(H * DK // 128):
                            nc.tensor.matmul(y2, G_sb[ct][:, qs * 128:(qs + 1) * 128], wo2[ct],
                                             start=(ct == 0), stop=(ct == H * DK // 128 - 1))
                        y2b = pc.tile([128, 512], F32, tag="y2b", bufs=3)
                        nc.vector.tensor_tensor(y2b, y2, bo_bc[:, o2c:o2c + 512], AL.add)
                        t2 = pc.tile([128, 512], F32, tag="t2", bufs=3)
                        nc.scalar.activation(t2, y2b, AF.Tanh, scale=0.5)
                        y1b = pc.tile([128, 512], F32, tag="y1b", bufs=3)
                        nc.vector.scalar_tensor_tensor(y1b, y1, 0.5, bo1h[:, o1c:o1c + 512],
                                                       op0=AL.mult, op1=AL.add)
                        oc = pc.tile([128, 512], F32, tag="oc", bufs=4)
                        nc.vector.scalar_tensor_tensor(oc, t2, 1.0, y1b,
                                                       op0=AL.add, op1=AL.mult)
                        nc.sync.dma_start(out=out[qs * 128:(qs + 1) * 128, o1c:o1c + 512], in_=oc)

    nc.compile()
    return nc


_NC = None


def kernel(**inputs):
    global _NC
    x = np.ascontiguousarray(np.asarray(inputs["x"], dtype=np.float32))
    mask = np.asarray(inputs["mask"])
    W_q = np.ascontiguousarray(np.asarray(inputs["W_q"], dtype=np.float32))
    W_k = np.ascontiguousarray(np.asarray(inputs["W_k"], dtype=np.float32))
    W_v = np.ascontiguousarray(np.asarray(inputs["W_v"], dtype=np.float32))
    W_o = np.ascontiguousarray(np.asarray(inputs["W_o"], dtype=np.float32))
    b_q = np.ascontiguousarray(np.asarray(inputs["b_q"], dtype=np.float32))
    b_k = np.ascontiguousarray(np.asarray(inputs["b_k"], dtype=np.float32))
    b_v = np.ascontiguousarray(np.asarray(inputs["b_v"], dtype=np.float32))
    b_o = np.ascontiguousarray(np.asarray(inputs["b_o"], dtype=np.float32))

    if _NC is None:
        _NC = _build()

    B = x.shape[0]
    in_maps = []
    for core in range(N_CORES):
        b, g = core // 2, core % 2
        xT = np.ascontiguousarray(x[b].T)
        in_maps.append({
            "xT": xT,
            "xTq": np.ascontiguousarray(xT[:, g * QH:(g + 1) * QH]),
            "wq": W_q, "wk": W_k, "wv": W_v, "wo": W_o,
            "bq": b_q, "bk": b_k, "bv": b_v, "bo": b_o,
            "maskf": np.ascontiguousarray(mask[b].astype(np.float32)),
        })

    res = run_bass_kernel_spmd(_NC, in_maps, core_ids=list(range(N_CORES)))
    out = np.empty((B, S, D), dtype=np.float32)
    for core in range(N_CORES):
        b, g = core // 2, core % 2
        out[b, g * QH:(g + 1) * QH, :] = res.results[core]["out"]
    return out


# revision 9
# speedup vs baseline: 1.0369x; 1.0029x over previous
"""Trainium2 Bass kernel for an 8-head GLU multi-head self-attention block.

Shapes (hardcoded from the problem spec):
  x [4, 2048, 1024], mask [4, 2048] (int32),
  W_q/W_k [1024, 2048], W_v [1024, 4096], W_o [2048, 2048],
  b_q/b_k [2048], b_v [4096], b_o [2048]  ->  out [4, 2048, 1024] f32.

Sharding: 8 cores = 4 batches x 2 query-halves. Each core computes K/V
projections for its full batch (duplicated within the pair - keeps the
program collective-free and fully static for SPMD), Q projection +
attention + output projection + GLUs for its 1024-query half, all 8 heads.

All matmuls run as float32r (full PE rate at N>=512, ~1e-4 rounding).
Layouts keep the contraction dim on SBUF partitions throughout:
  QT/KT [dk, q], scores transposed [k, q] (exp'd on ACT), V natural [k, dv]
so attention needs no on-chip transposes. Softmax denominator comes from a
mask-column matmul over the exp'd scores; sigmoid is computed via tanh to
stay inside the single ACT table set (exp/tanh/copy).
"""

import sys
import numpy as np

for _p in ("/opt/trn_rl_repo", "/root/.axon_site/_ro/trn_rl_repo"):
    if _p not in sys.path:
        sys.path.insert(0, _p)

import concourse.bass as bass
import concourse.mybir as mybir
import concourse.tile as tile
from concourse import bacc
from concourse.bass_utils import run_bass_kernel_spmd

F32 = mybir.dt.float32
F32R = mybir.dt.float32r
AL = mybir.AluOpType
AF = mybir.ActivationFunctionType

N_CORES = 8
S = 2048          # sequence length
D = 1024          # d_model
H = 8             # heads
DK = 256          # per-head q/k dim
DV = 512          # per-head v dim (GLU-doubled)
DO = 2048         # output-projection dim (GLU-doubled)
QH = S // 2       # queries per core


def _bcast_ap(vec_ap, parts, offset, n):
    """AP reading vec[offset:offset+n] broadcast across `parts` partitions."""
    return bass.AP(tensor=vec_ap.tensor, offset=offset, ap=[[0, parts], [1, n]])


def _build():
    nc = bacc.Bacc("TRN2", target_bir_lowering=False, debug=False,
                   num_devices=N_CORES)

    xT = nc.dram_tensor("xT", [D, S], F32, kind="ExternalInput").ap()
    xTq = nc.dram_tensor("xTq", [D, QH], F32, kind="ExternalInput").ap()
    wq = nc.dram_tensor("wq", [D, H * DK], F32, kind="ExternalInput").ap()
    wk = nc.dram_tensor("wk", [D, H * DK], F32, kind="ExternalInput").ap()
    wv = nc.dram_tensor("wv", [D, H * DV], F32, kind="ExternalInput").ap()
    wo = nc.dram_tensor("wo", [H * DK, DO], F32, kind="ExternalInput").ap()
    bq = nc.dram_tensor("bq", [H * DK], F32, kind="ExternalInput").ap()
    bk = nc.dram_tensor("bk", [H * DK], F32, kind="ExternalInput").ap()
    bv = nc.dram_tensor("bv", [H * DV], F32, kind="ExternalInput").ap()
    bo = nc.dram_tensor("bo", [DO], F32, kind="ExternalInput").ap()
    maskf = nc.dram_tensor("maskf", [S], F32, kind="ExternalInput").ap()
    out = nc.dram_tensor("out", [QH, D], F32, kind="ExternalOutput").ap()

    # DRAM spill for projections (written pass 1, streamed back pass 2/C).
    # Per-head tensors so pass-2 prefetch of head h doesn't wait on head h+1
    # writes (dependency tracking is per-tensor).
    QT_ds = [nc.dram_tensor(f"QT_d{h}", [DK, QH], F32).ap().bitcast(F32R) for h in range(H)]
    KT_ds = [nc.dram_tensor(f"KT_d{h}", [DK, S], F32).ap().bitcast(F32R) for h in range(H)]
    V_ds = [nc.dram_tensor(f"V_d{h}", [S, DV], F32).ap().bitcast(F32R) for h in range(H)]
    G_ds = [nc.dram_tensor(f"G_d{h}", [DK, QH], F32).ap().bitcast(F32R) for h in range(H)]

    with tile.TileContext(nc) as tc:
        with tc.tile_pool(name="consts", bufs=1) as consts:
            # Per-c-tile bias columns (f32: used as tensor_scalar operands).
            bq_cols = consts.tile([128, H * DK // 128], F32)
            bk_cols = consts.tile([128, H * DK // 128], F32)
            for ct in range(H * DK // 128):
                nc.sync.dma_start(out=bq_cols[:, ct:ct + 1],
                                  in_=bq[ct * 128:(ct + 1) * 128].rearrange("(p o) -> p o", o=1))
                nc.sync.dma_start(out=bk_cols[:, ct:ct + 1],
                                  in_=bk[ct * 128:(ct + 1) * 128].rearrange("(p o) -> p o", o=1))
            # Mask columns per k-tile: f32 (scalar operand), x0.5 variant, f32r (matmul lhsT).
            mcol = consts.tile([128, S // 128], F32)
            for kt in range(S // 128):
                nc.sync.dma_start(out=mcol[:, kt:kt + 1],
                                  in_=maskf[kt * 128:(kt + 1) * 128].rearrange("(p o) -> p o", o=1))
            mhalf = consts.tile([128, S // 128], F32)
            nc.vector.tensor_scalar_mul(mhalf, mcol, 0.5)
            mcol_r = consts.tile([128, S // 128], F32R)
            nc.vector.tensor_copy(mcol_r, mcol)
            # Row of ones (bc matmul lhsT, K=1).
            ones_f = consts.tile([1, 128], F32)
            nc.vector.memset(ones_f, 1.0)
            ones1 = consts.tile([1, 128], F32R)
            nc.vector.tensor_copy(ones1, ones_f)
            ones_c = consts.tile([128, 1], F32)
            nc.vector.memset(ones_c, 1.0)
            ones128 = consts.tile([128, 1], F32R)
            nc.vector.tensor_copy(ones128, ones_c)
            # b_o broadcast [128, DO]; plus 0.5*b_o[:D] for the final GLU.
            bo_bc = consts.tile([128, DO], F32)
            nc.gpsimd.dma_start(out=bo_bc, in_=_bcast_ap(bo, 128, 0, DO))
            bo1h = consts.tile([128, D], F32)
            nc.vector.tensor_scalar_mul(bo1h, bo_bc[:, 0:D], 0.5)

            # ---------------- Pass 1: QKV projections -> DRAM ----------------
            with tc.tile_pool(name="p1", bufs=2) as p1, \
                 tc.tile_pool(name="ps1", bufs=2, space="PSUM") as ps1:
                xT_sb = []
                for d in range(D // 128):
                    t = p1.tile([128, S], F32R, tag="xT_sb", bufs=8)
                    nc.sync.dma_start(out=t, in_=xT[d * 128:(d + 1) * 128, :].bitcast(F32R))
                    xT_sb.append(t)
                xTq_sb = []
                for d in range(D // 128):
                    t = p1.tile([128, QH], F32R, tag="xTq_sb", bufs=8)
                    nc.sync.dma_start(out=t, in_=xTq[d * 128:(d + 1) * 128, :].bitcast(F32R))
                    xTq_sb.append(t)

                for h in range(H):
                    # K^T and Q^T: [dk, seq] via lhsT=W-block [d,128c], rhs=xT [d, seq-chunk]
                    for ct in range(2):
                        c0 = h * DK + ct * 128
                        wkb = []
                        wqb = []
                        for d in range(D // 128):
                            tk = p1.tile([128, 128], F32R, tag="wkb", bufs=24)
                            nc.sync.dma_start(out=tk, in_=wk[d * 128:(d + 1) * 128, c0:c0 + 128].bitcast(F32R))
                            wkb.append(tk)
                            tq = p1.tile([128, 128], F32R, tag="wqb", bufs=24)
                            nc.sync.dma_start(out=tq, in_=wq[d * 128:(d + 1) * 128, c0:c0 + 128].bitcast(F32R))
                            wqb.append(tq)
                        for kc in range(S // 512):
                            ps = ps1.tile([128, 512], F32, tag="ps1")
                            for d in range(D // 128):
                                nc.tensor.matmul(ps, wkb[d], xT_sb[d][:, kc * 512:(kc + 1) * 512],
                                                 start=(d == 0), stop=(d == D // 128 - 1))
                            ev = p1.tile([128, 512], F32R, tag="ev", bufs=8)
                            nc.vector.tensor_scalar(ev, ps, bk_cols[:, h * 2 + ct:h * 2 + ct + 1],
                                                    None, op0=AL.add)
                            nc.sync.dma_start(out=KT_ds[h][ct * 128:ct * 128 + 128, kc * 512:(kc + 1) * 512], in_=ev)
                        for qc in range(QH // 512):
                            ps = ps1.tile([128, 512], F32, tag="ps1")
                            for d in range(D // 128):
                                nc.tensor.matmul(ps, wqb[d], xTq_sb[d][:, qc * 512:(qc + 1) * 512],
                                                 start=(d == 0), stop=(d == D // 128 - 1))
                            ev = p1.tile([128, 512], F32R, tag="ev", bufs=8)
                            nc.vector.tensor_scalar(ev, ps, bq_cols[:, h * 2 + ct:h * 2 + ct + 1],
                                                    0.0625, op0=AL.add, op1=AL.mult)
                            nc.sync.dma_start(out=QT_ds[h][ct * 128:ct * 128 + 128, qc * 512:(qc + 1) * 512], in_=ev)
                    # V natural [k, dv]: lhsT=xT-block [d, 128k], rhs=wv [d, 512dv]
                    v0 = h * DV
                    wvb = []
                    for d in range(D // 128):
                        t = p1.tile([128, DV], F32R, tag="wvb", bufs=16)
                        nc.sync.dma_start(out=t, in_=wv[d * 128:(d + 1) * 128, v0:v0 + DV].bitcast(F32R))
                        wvb.append(t)
                    bva = p1.tile([128, 256], F32, tag="bva", bufs=4)
                    nc.gpsimd.dma_start(out=bva, in_=_bcast_ap(bv, 128, v0, 256))
                    bvah = p1.tile([128, 256], F32, tag="bvah", bufs=4)
                    nc.vector.tensor_scalar_mul(bvah, bva, 0.5)
                    bvg = p1.tile([128, 256], F32, tag="bvg", bufs=4)
                    nc.gpsimd.dma_start(out=bvg, in_=_bcast_ap(bv, 128, v0 + 256, 256))
                    for kt in range(S // 128):
                        ps = ps1.tile([128, 512], F32, tag="ps1")
                        for d in range(D // 128):
                            nc.tensor.matmul(ps, xT_sb[d][:, kt * 128:(kt + 1) * 128], wvb[d],
                                             start=(d == 0), stop=(d == D // 128 - 1))
                        ev = p1.tile([128, 512], F32R, tag="ev", bufs=8)
                        # a-half gets the extra 0.5 of the GLU-sigmoid identity folded in
                        nc.vector.scalar_tensor_tensor(ev[:, 0:256], ps[:, 0:256],
                                                       mhalf[:, kt:kt + 1], bvah,
                                                       op0=AL.mult, op1=AL.add)
                        nc.vector.scalar_tensor_tensor(ev[:, 256:512], ps[:, 256:512],
                                                       mcol[:, kt:kt + 1], bvg,
                                                       op0=AL.mult, op1=AL.add)
                        nc.sync.dma_start(out=V_ds[h][kt * 128:(kt + 1) * 128, :], in_=ev)

            # ---------------- Pass 2: attention per head ----------------
            with tc.tile_pool(name="p2", bufs=2) as p2, \
                 tc.tile_pool(name="ps_st", bufs=3, space="PSUM") as ps_st, \
                 tc.tile_pool(name="ps_ot", bufs=4, space="PSUM") as ps_ot, \
                 tc.tile_pool(name="ps_dn", bufs=1, space="PSUM") as ps_dn:
                pending_tail = None
                for h in range(H):
                    QT_h = []
                    KT_h = []
                    for ct in range(2):
                        c0 = h * DK + ct * 128
                        tq = p2.tile([128, QH], F32R, tag="qt", bufs=4)
                        nc.sync.dma_start(out=tq, in_=QT_ds[h][ct * 128:ct * 128 + 128, :])
                        QT_h.append(tq)
                        tk = p2.tile([128, S], F32R, tag="kt", bufs=4)
                        nc.sync.dma_start(out=tk, in_=KT_ds[h][ct * 128:ct * 128 + 128, :])
                        KT_h.append(tk)
                    V_h = []
                    for kt in range(S // 128):
                        tv = p2.tile([128, DV], F32R, tag="vt", bufs=24)
                        nc.sync.dma_start(out=tv, in_=V_ds[h][kt * 128:(kt + 1) * 128, :])
                        V_h.append(tv)
                    for qc in range(QH // 512):
                        q0 = qc * 512
                        ET = []
                        for kt in range(S // 128):
                            st = ps_st.tile([128, 512], F32, tag="st")
                            nc.tensor.matmul(st, KT_h[0][:, kt * 128:(kt + 1) * 128],
                                             QT_h[0][:, q0:q0 + 512], start=True, stop=False)
                            nc.tensor.matmul(st, KT_h[1][:, kt * 128:(kt + 1) * 128],
                                             QT_h[1][:, q0:q0 + 512], start=False, stop=True)
                            e = p2.tile([128, 512], F32R, tag="et", bufs=28)
                            nc.scalar.activation(e, st, AF.Exp)
                            ET.append(e)
                        # tail of the previous (h, qc) runs here: its ACT/DVE work is
                        # ready by now, and it frees the ot psum slots before this
                        # iteration's AV matmuls need them.
                        if pending_tail is not None:
                            pending_tail()
                            pending_tail = None
                        ots = [ps_ot.tile([128, 512], F32, tag="ot", name=f"ot{_i}") for _i in range(4)]
                        den = ps_dn.tile([1, 512], F32, tag="den")
                        for kt in range(S // 128):
                            first, last = kt == 0, kt == S // 128 - 1
                            for dvt in range(4):
                                nc.tensor.matmul(ots[dvt], V_h[kt][:, dvt * 128:(dvt + 1) * 128],
                                                 ET[kt], start=first, stop=last)
                            nc.tensor.matmul(den, mcol_r[:, kt:kt + 1], ET[kt],
                                             start=first, stop=last)
                        dsb = p2.tile([1, 512], F32R, tag="dsb", bufs=2)
                        nc.vector.tensor_copy(dsb, den)
                        bcp = ps_dn.tile([128, 512], F32, tag="den")
                        nc.tensor.matmul(bcp, ones1, dsb, start=True, stop=True)
                        bc = p2.tile([128, 512], F32, tag="bc", bufs=2)
                        with nc.allow_low_precision(reason="recip of broadcast denominator"):
                            nc.vector.reciprocal(bc, bcp)

                        def _tail(h=h, q0=q0, ots=ots, bc=bc):
                            for c2 in range(2):
                                an = p2.tile([128, 512], F32, tag="an", bufs=2, name="an")
                                nc.vector.tensor_tensor(an, ots[c2], bc, AL.mult)
                                gn = p2.tile([128, 512], F32, tag="gn", bufs=2, name="gn")
                                nc.vector.tensor_tensor(gn, ots[2 + c2], bc, AL.mult)
                                tg = p2.tile([128, 512], F32, tag="tg", bufs=2, name="tg")
                                nc.scalar.activation(tg, gn, AF.Tanh, scale=0.5)
                                go = p2.tile([128, 512], F32R, tag="go", bufs=4, name="go")
                                nc.vector.scalar_tensor_tensor(go, tg, 1.0, an,
                                                               op0=AL.add, op1=AL.mult)
                                nc.sync.dma_start(out=G_ds[h][c2 * 128:(c2 + 1) * 128,
                                                              q0:q0 + 512], in_=go)
                        pending_tail = _tail

                if pending_tail is not None:
                    pending_tail()
                    pending_tail = None

            # ---------------- Phase C: output projection + GLU ----------------
            with tc.tile_pool(name="pc", bufs=2) as pc, \
                 tc.tile_pool(name="ps_y", bufs=4, space="PSUM") as ps_y:
                G_sb = []
                for ct in range(H * DK // 128):
                    t = pc.tile([128, QH], F32R, tag="g_sb", bufs=16)
                    nc.sync.dma_start(out=t, in_=G_ds[ct // 2][(ct % 2) * 128:(ct % 2) * 128 + 128, :])
                    G_sb.append(t)
                for pair in range(2):
                    o1c = pair * 512
                    o2c = D + pair * 512
                    wo1 = []
                    wo2 = []
                    for ct in range(H * DK // 128):
                        t1 = pc.tile([128, 512], F32R, tag="wo1", bufs=16)
                        nc.sync.dma_start(out=t1, in_=wo[ct * 128:(ct + 1) * 128,
                                                        o1c:o1c + 512].bitcast(F32R))
                        wo1.append(t1)
                        t2 = pc.tile([128, 512], F32R, tag="wo2", bufs=16)
                        nc.sync.dma_start(out=t2, in_=wo[ct * 128:(ct + 1) * 128,
                                                        o2c:o2c + 512].bitcast(F32R))
                        wo2.append(t2)
                    for qs in range(QH // 128):
                        y1 = ps_y.tile([128, 512], F32, tag="y")
                        y2 = ps_y.tile([128, 512], F32, tag="y")
                        for ct in range(H * DK // 128):
                            nc.tensor.matmul(y1, G_sb[ct][:, qs * 128:(qs + 1) * 128], wo1[ct],
                                             start=(ct == 0), stop=(ct == H * DK // 128 - 1))
                        for ct in range(H * DK // 128):
                            nc.tensor.matmul(y2, G_sb[ct][:, qs * 128:(qs + 1) * 128], wo2[ct],
                                             start=(ct == 0), stop=(ct == H * DK // 128 - 1))
                        y2b = pc.tile([128, 512], F32, tag="y2b", bufs=3)
                        nc.vector.tensor_tensor(y2b, y2, bo_bc[:, o2c:o2c + 512], AL.add)
                        t2 = pc.tile([128, 512], F32, tag="t2", bufs=3)
                        nc.scalar.activation(t2, y2b, AF.Tanh, scale=0.5)
                        y1b = pc.tile([128, 512], F32, tag="y1b", bufs=3)
                        nc.vector.scalar_tensor_tensor(y1b, y1, 0.5, bo1h[:, o1c:o1c + 512],
                                                       op0=AL.mult, op1=AL.add)
                        oc = pc.tile([128, 512], F32, tag="oc", bufs=4)
                        nc.vector.scalar_tensor_tensor(oc, t2, 1.0, y1b,
                                                       op0=AL.add, op1=AL.mult)
                        nc.sync.dma_start(out=out[qs * 128:(qs + 1) * 128, o1c:o1c + 512], in_=oc)

    nc.compile()
    return nc


_NC = None


def kernel(**inputs):
    global _NC
    x = np.ascontiguousarray(np.asarray(inputs["x"], dtype=np.float32))
    mask = np.asarray(inputs["mask"])
    W_q = np.ascontiguousarray(np.asarray(inputs["W_q"], dtype=np.float32))
    W_k = np.ascontiguousarray(np.asarray(inputs["W_k"], dtype=np.float32))
    W_v = np.ascontiguousarray(np.asarray(inputs["W_v"], dtype=np.float32))
    W_o = np.ascontiguousarray(np.asarray(inputs["W_o"], dtype=np.float32))
    b_q = np.ascontiguousarray(np.asarray(inputs["b_q"], dtype=np.float32))
    b_k = np.ascontiguousarray(np.asarray(inputs["b_k"], dtype=np.float32))
    b_v = np.ascontiguousarray(np.asarray(inputs["b_v"], dtype=np.float32))
    b_o = np.ascontiguousarray(np.asarray(inputs["b_o"], dtype=np.float32))

    if _NC is None:
        _NC = _build()

    B = x.shape[0]
    in_maps = []
    for core in range(N_CORES):
        b, g = core // 2, core % 2
        xT = np.ascontiguousarray(x[b].T)
        in_maps.append({
            "xT": xT,
            "xTq": np.ascontiguousarray(xT[:, g * QH:(g + 1) * QH]),
            "wq": W_q, "wk": W_k, "wv": W_v, "wo": W_o,
            "bq": b_q, "bk": b_k, "bv": b_v, "bo": b_o,
            "maskf": np.ascontiguousarray(mask[b].astype(np.float32)),
        })

    res = run_bass_kernel_spmd(_NC, in_maps, core_ids=list(range(N_CORES)))
    out = np.empty((B, S, D), dtype=np.float32)
    for core in range(N_CORES):
        b, g = core // 2, core % 2
        out[b, g * QH:(g + 1) * QH, :] = res.results[core]["out"]
    return out


# revision 10
# speedup vs baseline: 1.0748x; 1.0366x over previous
"""Trainium2 Bass kernel for an 8-head GLU multi-head self-attention block.

Shapes (hardcoded from the problem spec):
  x [4, 2048, 1024], mask [4, 2048] (int32),
  W_q/W_k [1024, 2048], W_v [1024, 4096], W_o [2048, 2048],
  b_q/b_k [2048], b_v [4096], b_o [2048]  ->  out [4, 2048, 1024] f32.

Sharding: 8 cores = 4 batches x 2 query-halves. Each core computes K/V
projections for its full batch (duplicated within the pair - keeps the
program collective-free and fully static for SPMD), Q projection +
attention + output projection + GLUs for its 1024-query half, all 8 heads.

All matmuls run as float32r (full PE rate at N>=512, ~1e-4 rounding).
Layouts keep the contraction dim on SBUF partitions throughout:
  QT/KT [dk, q], scores transposed [k, q] (exp'd on ACT), V natural [k, dv]
so attention needs no on-chip transposes. Softmax denominator comes from a
mask-column matmul over the exp'd scores; sigmoid is computed via tanh to
stay inside the single ACT table set (exp/tanh/copy).
"""

import sys
import numpy as np

for _p in ("/opt/trn_rl_repo", "/root/.axon_site/_ro/trn_rl_repo"):
    if _p not in sys.path:
        sys.path.insert(0, _p)

import concourse.bass as bass
import concourse.mybir as mybir
import concourse.tile as tile
from concourse import bacc
from concourse.bass_utils import run_bass_kernel_spmd

F32 = mybir.dt.float32
F32R = mybir.dt.float32r
AL = mybir.AluOpType
AF = mybir.ActivationFunctionType

N_CORES = 8
S = 2048          # sequence length
D = 1024          # d_model
H = 8             # heads
DK = 256          # per-head q/k dim
DV = 512          # per-head v dim (GLU-doubled)
DO = 2048         # output-projection dim (GLU-doubled)
QH = S // 2       # queries per core


def _bcast_ap(vec_ap, parts, offset, n):
    """AP reading vec[offset:offset+n] broadcast across `parts` partitions."""
    return bass.AP(tensor=vec_ap.tensor, offset=offset, ap=[[0, parts], [1, n]])


def _build():
    nc = bacc.Bacc("TRN2", target_bir_lowering=False, debug=False,
                   num_devices=N_CORES)

    xT = nc.dram_tensor("xT", [D, S], F32, kind="ExternalInput").ap()
    xTq = nc.dram_tensor("xTq", [D, QH], F32, kind="ExternalInput").ap()
    wq = nc.dram_tensor("wq", [D, H * DK], F32, kind="ExternalInput").ap()
    wk = nc.dram_tensor("wk", [D, H * DK], F32, kind="ExternalInput").ap()
    wv = nc.dram_tensor("wv", [D, H * DV], F32, kind="ExternalInput").ap()
    wo = nc.dram_tensor("wo", [H * DK, DO], F32, kind="ExternalInput").ap()
    bq = nc.dram_tensor("bq", [H * DK], F32, kind="ExternalInput").ap()
    bk = nc.dram_tensor("bk", [H * DK], F32, kind="ExternalInput").ap()
    bv = nc.dram_tensor("bv", [H * DV], F32, kind="ExternalInput").ap()
    bo = nc.dram_tensor("bo", [DO], F32, kind="ExternalInput").ap()
    maskf = nc.dram_tensor("maskf", [S], F32, kind="ExternalInput").ap()
    out = nc.dram_tensor("out", [QH, D], F32, kind="ExternalOutput").ap()

    # DRAM spill for projections (written pass 1, streamed back pass 2/C).
    # Per-head tensors so pass-2 prefetch of head h doesn't wait on head h+1
    # writes (dependency tracking is per-tensor).
    QT_ds = [nc.dram_tensor(f"QT_d{h}", [DK, QH], F32).ap().bitcast(F32R) for h in range(H)]
    KT_ds = [nc.dram_tensor(f"KT_d{h}", [DK, S], F32).ap().bitcast(F32R) for h in range(H)]
    V_ds = [nc.dram_tensor(f"V_d{h}", [S, DV], F32).ap().bitcast(F32R) for h in range(H)]
    G_ds = [nc.dram_tensor(f"G_d{h}", [DK, QH], F32).ap().bitcast(F32R) for h in range(H)]

    with tile.TileContext(nc) as tc:
        with tc.tile_pool(name="consts", bufs=1) as consts:
            # Per-c-tile bias columns (f32: used as tensor_scalar operands).
            bq_cols = consts.tile([128, H * DK // 128], F32)
            bk_cols = consts.tile([128, H * DK // 128], F32)
            for ct in range(H * DK // 128):
                nc.sync.dma_start(out=bq_cols[:, ct:ct + 1],
                                  in_=bq[ct * 128:(ct + 1) * 128].rearrange("(p o) -> p o", o=1))
                nc.sync.dma_start(out=bk_cols[:, ct:ct + 1],
                                  in_=bk[ct * 128:(ct + 1) * 128].rearrange("(p o) -> p o", o=1))
            # Mask columns per k-tile: f32 (scalar operand), x0.5 variant, f32r (matmul lhsT).
            mcol = consts.tile([128, S // 128], F32)
            for kt in range(S // 128):
                nc.sync.dma_start(out=mcol[:, kt:kt + 1],
                                  in_=maskf[kt * 128:(kt + 1) * 128].rearrange("(p o) -> p o", o=1))
            mhalf = consts.tile([128, S // 128], F32)
            nc.vector.tensor_scalar_mul(mhalf, mcol, 0.5)
            mcol_r = consts.tile([128, S // 128], F32R)
            nc.vector.tensor_copy(mcol_r, mcol)
            # Row of ones (bc matmul lhsT, K=1).
            ones_f = consts.tile([1, 128], F32)
            nc.vector.memset(ones_f, 1.0)
            ones1 = consts.tile([1, 128], F32R)
            nc.vector.tensor_copy(ones1, ones_f)
            ones_c = consts.tile([128, 1], F32)
            nc.vector.memset(ones_c, 1.0)
            ones128 = consts.tile([128, 1], F32R)
            nc.vector.tensor_copy(ones128, ones_c)
            # b_o broadcast [128, DO]; plus 0.5*b_o[:D] for the final GLU.
            bo_bc = consts.tile([128, DO], F32)
            nc.gpsimd.dma_start(out=bo_bc, in_=_bcast_ap(bo, 128, 0, DO))
            bo1h = consts.tile([128, D], F32)
            nc.vector.tensor_scalar_mul(bo1h, bo_bc[:, 0:D], 0.5)

            # ---------------- Pass 1: QKV projections -> DRAM ----------------
            with tc.tile_pool(name="p1", bufs=2) as p1, \
                 tc.tile_pool(name="ps1", bufs=2, space="PSUM") as ps1:
                xT_sb = []
                for d in range(D // 128):
                    t = p1.tile([128, S], F32R, tag="xT_sb", bufs=8)
                    nc.sync.dma_start(out=t, in_=xT[d * 128:(d + 1) * 128, :].bitcast(F32R))
                    xT_sb.append(t)
                xTq_sb = []
                for d in range(D // 128):
                    t = p1.tile([128, QH], F32R, tag="xTq_sb", bufs=8)
                    nc.sync.dma_start(out=t, in_=xTq[d * 128:(d + 1) * 128, :].bitcast(F32R))
                    xTq_sb.append(t)

                for h in range(H):
                    # K^T and Q^T: [dk, seq] via lhsT=W-block [d,128c], rhs=xT [d, seq-chunk]
                    for ct in range(2):
                        c0 = h * DK + ct * 128
                        wkb = []
                        wqb = []
                        for d in range(D // 128):
                            tk = p1.tile([128, 128], F32R, tag="wkb", bufs=24)
                            nc.sync.dma_start(out=tk, in_=wk[d * 128:(d + 1) * 128, c0:c0 + 128].bitcast(F32R))
                            wkb.append(tk)
                            tq = p1.tile([128, 128], F32R, tag="wqb", bufs=24)
                            nc.sync.dma_start(out=tq, in_=wq[d * 128:(d + 1) * 128, c0:c0 + 128].bitcast(F32R))
                            wqb.append(tq)
                        for kc in range(S // 512):
                            ps = ps1.tile([128, 512], F32, tag="ps1")
                            for d in range(D // 128):
                                nc.tensor.matmul(ps, wkb[d], xT_sb[d][:, kc * 512:(kc + 1) * 512],
                                                 start=(d == 0), stop=(d == D // 128 - 1))
                            ev = p1.tile([128, 512], F32R, tag="ev", bufs=8)
                            nc.vector.tensor_scalar(ev, ps, bk_cols[:, h * 2 + ct:h * 2 + ct + 1],
                                                    None, op0=AL.add)
                            nc.sync.dma_start(out=KT_ds[h][ct * 128:ct * 128 + 128, kc * 512:(kc + 1) * 512], in_=ev)
                        for qc in range(QH // 512):
                            ps = ps1.tile([128, 512], F32, tag="ps1")
                            for d in range(D // 128):
                                nc.tensor.matmul(ps, wqb[d], xTq_sb[d][:, qc * 512:(qc + 1) * 512],
                                                 start=(d == 0), stop=(d == D // 128 - 1))
                            ev = p1.tile([128, 512], F32R, tag="ev", bufs=8)
                            nc.vector.tensor_scalar(ev, ps, bq_cols[:, h * 2 + ct:h * 2 + ct + 1],
                                                    0.0625, op0=AL.add, op1=AL.mult)
                            nc.sync.dma_start(out=QT_ds[h][ct * 128:ct * 128 + 128, qc * 512:(qc + 1) * 512], in_=ev)
                    # V natural [k, dv]: lhsT=xT-block [d, 128k], rhs=wv [d, 512dv]
                    v0 = h * DV
                    wvb = []
                    for d in range(D // 128):
                        t = p1.tile([128, DV], F32R, tag="wvb", bufs=16)
                        nc.sync.dma_start(out=t, in_=wv[d * 128:(d + 1) * 128, v0:v0 + DV].bitcast(F32R))
                        wvb.append(t)
                    bva = p1.tile([128, 256], F32, tag="bva", bufs=4)
                    nc.gpsimd.dma_start(out=bva, in_=_bcast_ap(bv, 128, v0, 256))
                    bvah = p1.tile([128, 256], F32, tag="bvah", bufs=4)
                    nc.vector.tensor_scalar_mul(bvah, bva, 0.5)
                    bvg = p1.tile([128, 256], F32, tag="bvg", bufs=4)
                    nc.gpsimd.dma_start(out=bvg, in_=_bcast_ap(bv, 128, v0 + 256, 256))
                    for kt in range(S // 128):
                        ps = ps1.tile([128, 512], F32, tag="ps1")
                        for d in range(D // 128):
                            nc.tensor.matmul(ps, xT_sb[d][:, kt * 128:(kt + 1) * 128], wvb[d],
                                             start=(d == 0), stop=(d == D // 128 - 1))
                        ev = p1.tile([128, 512], F32R, tag="ev", bufs=8)
                        # a-half gets the extra 0.5 of the GLU-sigmoid identity folded in
                        nc.vector.scalar_tensor_tensor(ev[:, 0:256], ps[:, 0:256],
                                                       mhalf[:, kt:kt + 1], bvah,
                                                       op0=AL.mult, op1=AL.add)
                        nc.vector.scalar_tensor_tensor(ev[:, 256:512], ps[:, 256:512],
                                                       mcol[:, kt:kt + 1], bvg,
                                                       op0=AL.mult, op1=AL.add)
                        nc.sync.dma_start(out=V_ds[h][kt * 128:(kt + 1) * 128, :], in_=ev)

            # ---------------- Pass 2: attention per head ----------------
            with tc.tile_pool(name="p2", bufs=2) as p2, \
                 tc.tile_pool(name="ps_st", bufs=3, space="PSUM") as ps_st, \
                 tc.tile_pool(name="ps_ot", bufs=4, space="PSUM") as ps_ot, \
                 tc.tile_pool(name="ps_dn", bufs=1, space="PSUM") as ps_dn:
                pending_tail = None
                for h in range(H):
                    QT_h = []
                    KT_h = []
                    for ct in range(2):
                        c0 = h * DK + ct * 128
                        tq = p2.tile([128, QH], F32R, tag="qt", bufs=4)
                        nc.sync.dma_start(out=tq, in_=QT_ds[h][ct * 128:ct * 128 + 128, :])
                        QT_h.append(tq)
                        tk = p2.tile([128, S], F32R, tag="kt", bufs=4)
                        nc.sync.dma_start(out=tk, in_=KT_ds[h][ct * 128:ct * 128 + 128, :])
                        KT_h.append(tk)
                    V_h = []
                    for kt in range(S // 128):
                        tv = p2.tile([128, DV], F32R, tag="vt", bufs=24)
                        nc.sync.dma_start(out=tv, in_=V_ds[h][kt * 128:(kt + 1) * 128, :])
                        V_h.append(tv)
                    for qc in range(QH // 512):
                        q0 = qc * 512
                        ET = []
                        for kt in range(S // 128):
                            st = ps_st.tile([128, 512], F32, tag="st")
                            nc.tensor.matmul(st, KT_h[0][:, kt * 128:(kt + 1) * 128],
                                             QT_h[0][:, q0:q0 + 512], start=True, stop=False)
                            nc.tensor.matmul(st, KT_h[1][:, kt * 128:(kt + 1) * 128],
                                             QT_h[1][:, q0:q0 + 512], start=False, stop=True)
                            e = p2.tile([128, 512], F32R, tag="et", bufs=28)
                            nc.scalar.activation(e, st, AF.Exp)
                            ET.append(e)
                        # tail of the previous (h, qc) runs here: its ACT/DVE work is
                        # ready by now, and it frees the ot psum slots before this
                        # iteration's AV matmuls need them.
                        if pending_tail is not None:
                            pending_tail()
                            pending_tail = None
                        ots = [ps_ot.tile([128, 512], F32, tag="ot", name=f"ot{_i}") for _i in range(4)]
                        den = ps_dn.tile([1, 512], F32, tag="den")
                        for kt in range(S // 128):
                            first, last = kt == 0, kt == S // 128 - 1
                            for dvt in range(4):
                                nc.tensor.matmul(ots[dvt], V_h[kt][:, dvt * 128:(dvt + 1) * 128],
                                                 ET[kt], start=first, stop=last)
                            nc.tensor.matmul(den, mcol_r[:, kt:kt + 1], ET[kt],
                                             start=first, stop=last)
                        dsb = p2.tile([1, 512], F32R, tag="dsb", bufs=2)
                        nc.vector.tensor_copy(dsb, den)
                        bcp = ps_dn.tile([128, 512], F32, tag="den")
                        nc.tensor.matmul(bcp, ones1, dsb, start=True, stop=True)
                        bc = p2.tile([128, 512], F32, tag="bc", bufs=2)
                        nc.vector.reciprocal_approx_fast(bc, bcp)

                        def _tail(h=h, q0=q0, ots=ots, bc=bc):
                            for c2 in range(2):
                                an = p2.tile([128, 512], F32, tag="an", bufs=2, name="an")
                                nc.vector.tensor_tensor(an, ots[c2], bc, AL.mult)
                                gn = p2.tile([128, 512], F32, tag="gn", bufs=2, name="gn")
                                nc.vector.tensor_tensor(gn, ots[2 + c2], bc, AL.mult)
                                tg = p2.tile([128, 512], F32, tag="tg", bufs=2, name="tg")
                                nc.scalar.activation(tg, gn, AF.Tanh, scale=0.5)
                                go = p2.tile([128, 512], F32R, tag="go", bufs=4, name="go")
                                nc.vector.scalar_tensor_tensor(go, tg, 1.0, an,
                                                               op0=AL.add, op1=AL.mult)
                                nc.sync.dma_start(out=G_ds[h][c2 * 128:(c2 + 1) * 128,
                                                              q0:q0 + 512], in_=go)
                        pending_tail = _tail

                if pending_tail is not None:
                    pending_tail()
                    pending_tail = None

            # ---------------- Phase C: output projection + GLU ----------------
            with tc.tile_pool(name="pc", bufs=2) as pc, \
                 tc.tile_pool(name="ps_y", bufs=4, space="PSUM") as ps_y:
                G_sb = []
                for ct in range(H * DK // 128):
                    t = pc.tile([128, QH], F32R, tag="g_sb", bufs=16)
                    nc.sync.dma_start(out=t, in_=G_ds[ct // 2][(ct % 2) * 128:(ct % 2) * 128 + 128, :])
                    G_sb.append(t)
                for pair in range(2):
                    o1c = pair * 512
                    o2c = D + pair * 512
                    wo1 = []
                    wo2 = []
                    for ct in range(H * DK // 128):
                        t1 = pc.tile([128, 512], F32R, tag="wo1", bufs=16)
                        nc.sync.dma_start(out=t1, in_=wo[ct * 128:(ct + 1) * 128,
                                                        o1c:o1c + 512].bitcast(F32R))
                        wo1.append(t1)
                        t2 = pc.tile([128, 512], F32R, tag="wo2", bufs=16)
                        nc.sync.dma_start(out=t2, in_=wo[ct * 128:(ct + 1) * 128,
                                                        o2c:o2c + 512].bitcast(F32R))
                        wo2.append(t2)
                    for qs in range(QH // 128):
                        y1 = ps_y.tile([128, 512], F32, tag="y")
                        y2 = ps_y.tile([128, 512], F32, tag="y")
                        for ct in range(H * DK // 128):
                            nc.tensor.matmul(y1, G_sb[ct][:, qs * 128:(qs + 1) * 128], wo1[ct],
                                             start=(ct == 0), stop=(ct == H * DK // 128 - 1))
                        for ct in range(H * DK // 128):
                            nc.tensor.matmul(y2, G_sb[ct][:, qs * 128:(qs + 1) * 128], wo2[ct],
                                             start=(ct == 0), stop=(ct == H * DK // 128 - 1))
                        y2b = pc.tile([128, 512], F32, tag="y2b", bufs=3)
                        nc.vector.tensor_tensor(y2b, y2, bo_bc[:, o2c:o2c + 512], AL.add)
                        t2 = pc.tile([128, 512], F32, tag="t2", bufs=3)
                        nc.scalar.activation(t2, y2b, AF.Tanh, scale=0.5)
                        y1b = pc.tile([128, 512], F32, tag="y1b", bufs=3)
                        nc.vector.scalar_tensor_tensor(y1b, y1, 0.5, bo1h[:, o1c:o1c + 512],
                                                       op0=AL.mult, op1=AL.add)
                        oc = pc.tile([128, 512], F32, tag="oc", bufs=4)
                        nc.vector.scalar_tensor_tensor(oc, t2, 1.0, y1b,
                                                       op0=AL.add, op1=AL.mult)
                        nc.sync.dma_start(out=out[qs * 128:(qs + 1) * 128, o1c:o1c + 512], in_=oc)

    nc.compile()
    return nc


_NC = None


def kernel(**inputs):
    global _NC
    x = np.ascontiguousarray(np.asarray(inputs["x"], dtype=np.float32))
    mask = np.asarray(inputs["mask"])
    W_q = np.ascontiguousarray(np.asarray(inputs["W_q"], dtype=np.float32))
    W_k = np.ascontiguousarray(np.asarray(inputs["W_k"], dtype=np.float32))
    W_v = np.ascontiguousarray(np.asarray(inputs["W_v"], dtype=np.float32))
    W_o = np.ascontiguousarray(np.asarray(inputs["W_o"], dtype=np.float32))
    b_q = np.ascontiguousarray(np.asarray(inputs["b_q"], dtype=np.float32))
    b_k = np.ascontiguousarray(np.asarray(inputs["b_k"], dtype=np.float32))
    b_v = np.ascontiguousarray(np.asarray(inputs["b_v"], dtype=np.float32))
    b_o = np.ascontiguousarray(np.asarray(inputs["b_o"], dtype=np.float32))

    if _NC is None:
        _NC = _build()

    B = x.shape[0]
    in_maps = []
    for core in range(N_CORES):
        b, g = core // 2, core % 2
        xT = np.ascontiguousarray(x[b].T)
        in_maps.append({
            "xT": xT,
            "xTq": np.ascontiguousarray(xT[:, g * QH:(g + 1) * QH]),
            "wq": W_q, "wk": W_k, "wv": W_v, "wo": W_o,
            "bq": b_q, "bk": b_k, "bv": b_v, "bo": b_o,
            "maskf": np.ascontiguousarray(mask[b].astype(np.float32)),
        })

    res = run_bass_kernel_spmd(_NC, in_maps, core_ids=list(range(N_CORES)))
    out = np.empty((B, S, D), dtype=np.float32)
    for core in range(N_CORES):
        b, g = core // 2, core % 2
        out[b, g * QH:(g + 1) * QH, :] = res.results[core]["out"]
    return out


# revision 11
# speedup vs baseline: 1.1092x; 1.0319x over previous
"""Trainium2 Bass kernel for an 8-head GLU multi-head self-attention block.

Shapes (hardcoded from the problem spec):
  x [4, 2048, 1024], mask [4, 2048] (int32),
  W_q/W_k [1024, 2048], W_v [1024, 4096], W_o [2048, 2048],
  b_q/b_k [2048], b_v [4096], b_o [2048]  ->  out [4, 2048, 1024] f32.

Sharding: 8 cores = 4 batches x 2 query-halves. Each core computes K/V
projections for its full batch (duplicated within the pair - keeps the
program collective-free and fully static for SPMD), Q projection +
attention + output projection + GLUs for its 1024-query half, all 8 heads.

All matmuls run as float32r (full PE rate at N>=512, ~1e-4 rounding).
Layouts keep the contraction dim on SBUF partitions throughout:
  QT/KT [dk, q], scores transposed [k, q] (exp'd on ACT), V natural [k, dv]
so attention needs no on-chip transposes. Softmax denominator comes from a
mask-column matmul over the exp'd scores; sigmoid is computed via tanh to
stay inside the single ACT table set (exp/tanh/copy).
"""

import sys
import numpy as np

for _p in ("/opt/trn_rl_repo", "/root/.axon_site/_ro/trn_rl_repo"):
    if _p not in sys.path:
        sys.path.insert(0, _p)

import concourse.bass as bass
import concourse.mybir as mybir
import concourse.tile as tile
from concourse import bacc
from concourse.bass_utils import run_bass_kernel_spmd

F32 = mybir.dt.float32
F32R = mybir.dt.float32r
AL = mybir.AluOpType
AF = mybir.ActivationFunctionType

N_CORES = 8
S = 2048          # sequence length
D = 1024          # d_model
H = 8             # heads
DK = 256          # per-head q/k dim
DV = 512          # per-head v dim (GLU-doubled)
DO = 2048         # output-projection dim (GLU-doubled)
QH = S // 2       # queries per core


def _bcast_ap(vec_ap, parts, offset, n):
    """AP reading vec[offset:offset+n] broadcast across `parts` partitions."""
    return bass.AP(tensor=vec_ap.tensor, offset=offset, ap=[[0, parts], [1, n]])


def _build():
    nc = bacc.Bacc("TRN2", target_bir_lowering=False, debug=False,
                   num_devices=N_CORES)

    xT = nc.dram_tensor("xT", [D, S], F32, kind="ExternalInput").ap()
    xTq = nc.dram_tensor("xTq", [D, QH], F32, kind="ExternalInput").ap()
    wq = nc.dram_tensor("wq", [D, H * DK], F32, kind="ExternalInput").ap()
    wk = nc.dram_tensor("wk", [D, H * DK], F32, kind="ExternalInput").ap()
    wv = nc.dram_tensor("wv", [D, H * DV], F32, kind="ExternalInput").ap()
    wo = nc.dram_tensor("wo", [H * DK, DO], F32, kind="ExternalInput").ap()
    bq = nc.dram_tensor("bq", [H * DK], F32, kind="ExternalInput").ap()
    bk = nc.dram_tensor("bk", [H * DK], F32, kind="ExternalInput").ap()
    bv = nc.dram_tensor("bv", [H * DV], F32, kind="ExternalInput").ap()
    bo = nc.dram_tensor("bo", [DO], F32, kind="ExternalInput").ap()
    maskf = nc.dram_tensor("maskf", [S], F32, kind="ExternalInput").ap()
    out = nc.dram_tensor("out", [QH, D], F32, kind="ExternalOutput").ap()

    # DRAM spill for projections (written pass 1, streamed back pass 2/C).
    # Per-head tensors so pass-2 prefetch of head h doesn't wait on head h+1
    # writes (dependency tracking is per-tensor).
    QT_ds = [nc.dram_tensor(f"QT_d{h}", [DK, QH], F32).ap().bitcast(F32R) for h in range(H)]
    KT_ds = [nc.dram_tensor(f"KT_d{h}", [DK, S], F32).ap().bitcast(F32R) for h in range(H)]
    V_ds = [nc.dram_tensor(f"V_d{h}", [S, DV], F32).ap().bitcast(F32R) for h in range(H)]
    G_ds = [nc.dram_tensor(f"G_d{h}", [DK, QH], F32).ap().bitcast(F32R) for h in range(H)]

    with tile.TileContext(nc) as tc:
        with tc.tile_pool(name="consts", bufs=1) as consts:
            # Per-c-tile bias columns (f32: used as tensor_scalar operands).
            bq_cols = consts.tile([128, H * DK // 128], F32)
            bk_cols = consts.tile([128, H * DK // 128], F32)
            for ct in range(H * DK // 128):
                nc.sync.dma_start(out=bq_cols[:, ct:ct + 1],
                                  in_=bq[ct * 128:(ct + 1) * 128].rearrange("(p o) -> p o", o=1))
                nc.sync.dma_start(out=bk_cols[:, ct:ct + 1],
                                  in_=bk[ct * 128:(ct + 1) * 128].rearrange("(p o) -> p o", o=1))
            # Mask columns per k-tile: f32 (scalar operand), x0.5 variant, f32r (matmul lhsT).
            mcol = consts.tile([128, S // 128], F32)
            for kt in range(S // 128):
                nc.sync.dma_start(out=mcol[:, kt:kt + 1],
                                  in_=maskf[kt * 128:(kt + 1) * 128].rearrange("(p o) -> p o", o=1))
            mhalf = consts.tile([128, S // 128], F32)
            nc.vector.tensor_scalar_mul(mhalf, mcol, 0.5)
            mcol_r = consts.tile([128, S // 128], F32R)
            nc.vector.tensor_copy(mcol_r, mcol)
            # Row of ones (bc matmul lhsT, K=1).
            ones_f = consts.tile([1, 128], F32)
            nc.vector.memset(ones_f, 1.0)
            ones1 = consts.tile([1, 128], F32R)
            nc.vector.tensor_copy(ones1, ones_f)
            ones_c = consts.tile([128, 1], F32)
            nc.vector.memset(ones_c, 1.0)
            ones128 = consts.tile([128, 1], F32R)
            nc.vector.tensor_copy(ones128, ones_c)
            # b_o broadcast [128, DO]; plus 0.5*b_o[:D] for the final GLU.
            bo_bc = consts.tile([128, DO], F32)
            nc.gpsimd.dma_start(out=bo_bc, in_=_bcast_ap(bo, 128, 0, DO))
            bo1h = consts.tile([128, D], F32)
            nc.vector.tensor_scalar_mul(bo1h, bo_bc[:, 0:D], 0.5)

            # ---------------- Pass 1: QKV projections -> DRAM ----------------
            with tc.tile_pool(name="p1", bufs=2) as p1, \
                 tc.tile_pool(name="ps1", bufs=2, space="PSUM") as ps1:
                # column-chunked so the first psum group only waits on 2MB
                xT_ch = {}
                xTq_ch = {}
                for kc in range(S // 512):
                    for d in range(D // 128):
                        t = p1.tile([128, 512], F32R, tag="xT_sb", bufs=32, name="xT_c")
                        nc.sync.dma_start(out=t, in_=xT[d * 128:(d + 1) * 128,
                                                       kc * 512:(kc + 1) * 512].bitcast(F32R))
                        xT_ch[(d, kc)] = t
                        if kc < QH // 512:
                            tq2_ = p1.tile([128, 512], F32R, tag="xTq_sb", bufs=16, name="xTq_c")
                            nc.sync.dma_start(out=tq2_, in_=xTq[d * 128:(d + 1) * 128,
                                                              kc * 512:(kc + 1) * 512].bitcast(F32R))
                            xTq_ch[(d, kc)] = tq2_

                for h in range(H):
                    # K^T and Q^T: [dk, seq] via lhsT=W-block [d,128c], rhs=xT [d, seq-chunk]
                    for ct in range(2):
                        c0 = h * DK + ct * 128
                        wkb = []
                        wqb = []
                        for d in range(D // 128):
                            tk = p1.tile([128, 128], F32R, tag="wkb", bufs=24)
                            nc.sync.dma_start(out=tk, in_=wk[d * 128:(d + 1) * 128, c0:c0 + 128].bitcast(F32R))
                            wkb.append(tk)
                            tq = p1.tile([128, 128], F32R, tag="wqb", bufs=24)
                            nc.sync.dma_start(out=tq, in_=wq[d * 128:(d + 1) * 128, c0:c0 + 128].bitcast(F32R))
                            wqb.append(tq)
                        for kc in range(S // 512):
                            ps = ps1.tile([128, 512], F32, tag="ps1")
                            for d in range(D // 128):
                                nc.tensor.matmul(ps, wkb[d], xT_ch[(d, kc)],
                                                 start=(d == 0), stop=(d == D // 128 - 1))
                            ev = p1.tile([128, 512], F32R, tag="ev", bufs=8)
                            nc.vector.tensor_scalar(ev, ps, bk_cols[:, h * 2 + ct:h * 2 + ct + 1],
                                                    None, op0=AL.add)
                            nc.sync.dma_start(out=KT_ds[h][ct * 128:ct * 128 + 128, kc * 512:(kc + 1) * 512], in_=ev)
                        for qc in range(QH // 512):
                            ps = ps1.tile([128, 512], F32, tag="ps1")
                            for d in range(D // 128):
                                nc.tensor.matmul(ps, wqb[d], xTq_ch[(d, qc)],
                                                 start=(d == 0), stop=(d == D // 128 - 1))
                            ev = p1.tile([128, 512], F32R, tag="ev", bufs=8)
                            nc.vector.tensor_scalar(ev, ps, bq_cols[:, h * 2 + ct:h * 2 + ct + 1],
                                                    0.0625, op0=AL.add, op1=AL.mult)
                            nc.sync.dma_start(out=QT_ds[h][ct * 128:ct * 128 + 128, qc * 512:(qc + 1) * 512], in_=ev)
                    # V natural [k, dv]: lhsT=xT-block [d, 128k], rhs=wv [d, 512dv]
                    v0 = h * DV
                    wvb = []
                    for d in range(D // 128):
                        t = p1.tile([128, DV], F32R, tag="wvb", bufs=16)
                        nc.sync.dma_start(out=t, in_=wv[d * 128:(d + 1) * 128, v0:v0 + DV].bitcast(F32R))
                        wvb.append(t)
                    bva = p1.tile([128, 256], F32, tag="bva", bufs=4)
                    nc.gpsimd.dma_start(out=bva, in_=_bcast_ap(bv, 128, v0, 256))
                    bvah = p1.tile([128, 256], F32, tag="bvah", bufs=4)
                    nc.vector.tensor_scalar_mul(bvah, bva, 0.5)
                    bvg = p1.tile([128, 256], F32, tag="bvg", bufs=4)
                    nc.gpsimd.dma_start(out=bvg, in_=_bcast_ap(bv, 128, v0 + 256, 256))
                    for kt in range(S // 128):
                        ps = ps1.tile([128, 512], F32, tag="ps1")
                        for d in range(D // 128):
                            nc.tensor.matmul(ps, xT_ch[(d, kt // 4)][:, (kt % 4) * 128:(kt % 4) * 128 + 128], wvb[d],
                                             start=(d == 0), stop=(d == D // 128 - 1))
                        ev = p1.tile([128, 512], F32R, tag="ev", bufs=8)
                        # a-half gets the extra 0.5 of the GLU-sigmoid identity folded in
                        nc.vector.scalar_tensor_tensor(ev[:, 0:256], ps[:, 0:256],
                                                       mhalf[:, kt:kt + 1], bvah,
                                                       op0=AL.mult, op1=AL.add)
                        nc.vector.scalar_tensor_tensor(ev[:, 256:512], ps[:, 256:512],
                                                       mcol[:, kt:kt + 1], bvg,
                                                       op0=AL.mult, op1=AL.add)
                        nc.sync.dma_start(out=V_ds[h][kt * 128:(kt + 1) * 128, :], in_=ev)

            # ---------------- Pass 2: attention per head ----------------
            with tc.tile_pool(name="p2", bufs=2) as p2, \
                 tc.tile_pool(name="ps_st", bufs=3, space="PSUM") as ps_st, \
                 tc.tile_pool(name="ps_ot", bufs=4, space="PSUM") as ps_ot, \
                 tc.tile_pool(name="ps_dn", bufs=1, space="PSUM") as ps_dn:
                pending_tail = None
                for h in range(H):
                    QT_h = []
                    KT_h = []
                    for ct in range(2):
                        c0 = h * DK + ct * 128
                        tq = p2.tile([128, QH], F32R, tag="qt", bufs=4)
                        nc.sync.dma_start(out=tq, in_=QT_ds[h][ct * 128:ct * 128 + 128, :])
                        QT_h.append(tq)
                        tk = p2.tile([128, S], F32R, tag="kt", bufs=4)
                        nc.sync.dma_start(out=tk, in_=KT_ds[h][ct * 128:ct * 128 + 128, :])
                        KT_h.append(tk)
                    V_h = []
                    for kt in range(S // 128):
                        tv = p2.tile([128, DV], F32R, tag="vt", bufs=24)
                        nc.sync.dma_start(out=tv, in_=V_ds[h][kt * 128:(kt + 1) * 128, :])
                        V_h.append(tv)
                    for qc in range(QH // 512):
                        q0 = qc * 512
                        # previous iteration's tail first: its inputs are ready and it
                        # must lead the DVE/ACT queues so ot psum slots free early.
                        if pending_tail is not None:
                            pending_tail()
                            pending_tail = None
                        ET = []
                        acc = None
                        for kt in range(S // 128):
                            st = ps_st.tile([128, 512], F32, tag="st")
                            nc.tensor.matmul(st, KT_h[0][:, kt * 128:(kt + 1) * 128],
                                             QT_h[0][:, q0:q0 + 512], start=True, stop=False)
                            nc.tensor.matmul(st, KT_h[1][:, kt * 128:(kt + 1) * 128],
                                             QT_h[1][:, q0:q0 + 512], start=False, stop=True)
                            e = p2.tile([128, 512], F32R, tag="et", bufs=28)
                            nc.scalar.activation(e, st, AF.Exp)
                            ET.append(e)
                            # masked-exp running sum on DVE (ping-pong, partition-wise)
                            nacc = p2.tile([128, 512], F32R, tag="acc", bufs=4, name="acc")
                            if acc is None:
                                nc.vector.tensor_scalar(nacc, e, mcol[:, kt:kt + 1],
                                                        None, op0=AL.mult)
                            else:
                                nc.vector.scalar_tensor_tensor(nacc, e, mcol[:, kt:kt + 1],
                                                               acc, op0=AL.mult, op1=AL.add)
                            acc = nacc
                        ots = [ps_ot.tile([128, 512], F32, tag="ot", name=f"ot{_i}") for _i in range(4)]
                        den = ps_dn.tile([1, 512], F32, tag="den")
                        nc.tensor.matmul(den, ones128, acc, start=True, stop=True)
                        dsb = p2.tile([1, 512], F32R, tag="dsb", bufs=2)
                        nc.vector.tensor_copy(dsb, den)
                        for kt in range(4):
                            for dvt in range(4):
                                nc.tensor.matmul(ots[dvt], V_h[kt][:, dvt * 128:(dvt + 1) * 128],
                                                 ET[kt], start=(kt == 0), stop=False)
                        bcp = ps_dn.tile([128, 512], F32, tag="den")
                        nc.tensor.matmul(bcp, ones1, dsb, start=True, stop=True)
                        bc = p2.tile([128, 512], F32, tag="bc", bufs=2)
                        nc.vector.reciprocal_approx_fast(bc, bcp)
                        for kt in range(4, S // 128):
                            for dvt in range(4):
                                nc.tensor.matmul(ots[dvt], V_h[kt][:, dvt * 128:(dvt + 1) * 128],
                                                 ET[kt], start=False, stop=(kt == S // 128 - 1))

                        def _tail(h=h, q0=q0, ots=ots, bc=bc):
                            for c2 in range(2):
                                an = p2.tile([128, 512], F32, tag="an", bufs=2, name="an")
                                nc.vector.tensor_tensor(an, ots[c2], bc, AL.mult)
                                gn = p2.tile([128, 512], F32, tag="gn", bufs=2, name="gn")
                                nc.vector.tensor_tensor(gn, ots[2 + c2], bc, AL.mult)
                                tg = p2.tile([128, 512], F32, tag="tg", bufs=2, name="tg")
                                nc.scalar.activation(tg, gn, AF.Tanh, scale=0.5)
                                go = p2.tile([128, 512], F32R, tag="go", bufs=4, name="go")
                                nc.vector.scalar_tensor_tensor(go, tg, 1.0, an,
                                                               op0=AL.add, op1=AL.mult)
                                nc.sync.dma_start(out=G_ds[h][c2 * 128:(c2 + 1) * 128,
                                                              q0:q0 + 512], in_=go)
                        pending_tail = _tail

                if pending_tail is not None:
                    pending_tail()
                    pending_tail = None

            # ---------------- Phase C: output projection + GLU ----------------
            with tc.tile_pool(name="pc", bufs=2) as pc, \
                 tc.tile_pool(name="ps_y", bufs=4, space="PSUM") as ps_y:
                G_sb = []
                for ct in range(H * DK // 128):
                    t = pc.tile([128, QH], F32R, tag="g_sb", bufs=16)
                    nc.sync.dma_start(out=t, in_=G_ds[ct // 2][(ct % 2) * 128:(ct % 2) * 128 + 128, :])
                    G_sb.append(t)
                for pair in range(2):
                    o1c = pair * 512
                    o2c = D + pair * 512
                    wo1 = []
                    wo2 = []
                    for ct in range(H * DK // 128):
                        t1 = pc.tile([128, 512], F32R, tag="wo1", bufs=16)
                        nc.sync.dma_start(out=t1, in_=wo[ct * 128:(ct + 1) * 128,
                                                        o1c:o1c + 512].bitcast(F32R))
                        wo1.append(t1)
                        t2 = pc.tile([128, 512], F32R, tag="wo2", bufs=16)
                        nc.sync.dma_start(out=t2, in_=wo[ct * 128:(ct + 1) * 128,
                                                        o2c:o2c + 512].bitcast(F32R))
                        wo2.append(t2)
                    for qs in range(QH // 128):
                        y1 = ps_y.tile([128, 512], F32, tag="y")
                        y2 = ps_y.tile([128, 512], F32, tag="y")
                        for ct in range(H * DK // 128):
                            nc.tensor.matmul(y1, G_sb[ct][:, qs * 128:(qs + 1) * 128], wo1[ct],
                                             start=(ct == 0), stop=(ct == H * DK // 128 - 1))
                        for ct in range(H * DK // 128):
                            nc.tensor.matmul(y2, G_sb[ct][:, qs * 128:(qs + 1) * 128], wo2[ct],
                                             start=(ct == 0), stop=(ct == H * DK // 128 - 1))
                        y2b = pc.tile([128, 512], F32, tag="y2b", bufs=3)
                        nc.vector.tensor_tensor(y2b, y2, bo_bc[:, o2c:o2c + 512], AL.add)
                        t2 = pc.tile([128, 512], F32, tag="t2", bufs=3)
                        nc.scalar.activation(t2, y2b, AF.Tanh, scale=0.5)
                        y1b = pc.tile([128, 512], F32, tag="y1b", bufs=3)
                        nc.vector.scalar_tensor_tensor(y1b, y1, 0.5, bo1h[:, o1c:o1c + 512],
                                                       op0=AL.mult, op1=AL.add)
                        oc = pc.tile([128, 512], F32, tag="oc", bufs=4)
                        nc.vector.scalar_tensor_tensor(oc, t2, 1.0, y1b,
                                                       op0=AL.add, op1=AL.mult)
                        nc.sync.dma_start(out=out[qs * 128:(qs + 1) * 128, o1c:o1c + 512], in_=oc)

    nc.compile()
    return nc


_NC = None


def kernel(**inputs):
    global _NC
    x = np.ascontiguousarray(np.asarray(inputs["x"], dtype=np.float32))
    mask = np.asarray(inputs["mask"])
    W_q = np.ascontiguousarray(np.asarray(inputs["W_q"], dtype=np.float32))
    W_k = np.ascontiguousarray(np.asarray(inputs["W_k"], dtype=np.float32))
    W_v = np.ascontiguousarray(np.asarray(inputs["W_v"], dtype=np.float32))
    W_o = np.ascontiguousarray(np.asarray(inputs["W_o"], dtype=np.float32))
    b_q = np.ascontiguousarray(np.asarray(inputs["b_q"], dtype=np.float32))
    b_k = np.ascontiguousarray(np.asarray(inputs["b_k"], dtype=np.float32))
    b_v = np.ascontiguousarray(np.asarray(inputs["b_v"], dtype=np.float32))
    b_o = np.ascontiguousarray(np.asarray(inputs["b_o"], dtype=np.float32))

    if _NC is None:
        _NC = _build()

    B = x.shape[0]
    in_maps = []
    for core in range(N_CORES):
        b, g = core // 2, core % 2
        xT = np.ascontiguousarray(x[b].T)
        in_maps.append({
            "xT": xT,
            "xTq": np.ascontiguousarray(xT[:, g * QH:(g + 1) * QH]),
            "wq": W_q, "wk": W_k, "wv": W_v, "wo": W_o,
            "bq": b_q, "bk": b_k, "bv": b_v, "bo": b_o,
            "maskf": np.ascontiguousarray(mask[b].astype(np.float32)),
        })

    res = run_bass_kernel_spmd(_NC, in_maps, core_ids=list(range(N_CORES)))
    out = np.empty((B, S, D), dtype=np.float32)
    for core in range(N_CORES):
        b, g = core // 2, core % 2
        out[b, g * QH:(g + 1) * QH, :] = res.results[core]["out"]
    return out


# revision 12
# speedup vs baseline: 1.1387x; 1.0266x over previous
"""Trainium2 Bass kernel for an 8-head GLU multi-head self-attention block.

Shapes (hardcoded from the problem spec):
  x [4, 2048, 1024], mask [4, 2048] (int32),
  W_q/W_k [1024, 2048], W_v [1024, 4096], W_o [2048, 2048],
  b_q/b_k [2048], b_v [4096], b_o [2048]  ->  out [4, 2048, 1024] f32.

Sharding: 8 cores = 4 batches x 2 query-halves. Each core computes K/V
projections for its full batch (duplicated within the pair - keeps the
program collective-free and fully static for SPMD), Q projection +
attention + output projection + GLUs for its 1024-query half, all 8 heads.

All matmuls run as float32r (full PE rate at N>=512, ~1e-4 rounding).
Layouts keep the contraction dim on SBUF partitions throughout:
  QT/KT [dk, q], scores transposed [k, q] (exp'd on ACT), V natural [k, dv]
so attention needs no on-chip transposes. Softmax denominator comes from a
mask-column matmul over the exp'd scores; sigmoid is computed via tanh to
stay inside the single ACT table set (exp/tanh/copy).
"""

import sys
import numpy as np

for _p in ("/opt/trn_rl_repo", "/root/.axon_site/_ro/trn_rl_repo"):
    if _p not in sys.path:
        sys.path.insert(0, _p)

import concourse.bass as bass
import concourse.mybir as mybir
import concourse.tile as tile
from concourse import bacc
from concourse.bass_utils import run_bass_kernel_spmd

F32 = mybir.dt.float32
F32R = mybir.dt.float32r
AL = mybir.AluOpType
AF = mybir.ActivationFunctionType

N_CORES = 8
S = 2048          # sequence length
D = 1024          # d_model
H = 8             # heads
DK = 256          # per-head q/k dim
DV = 512          # per-head v dim (GLU-doubled)
DO = 2048         # output-projection dim (GLU-doubled)
QH = S // 2       # queries per core


def _bcast_ap(vec_ap, parts, offset, n):
    """AP reading vec[offset:offset+n] broadcast across `parts` partitions."""
    return bass.AP(tensor=vec_ap.tensor, offset=offset, ap=[[0, parts], [1, n]])


def _build():
    nc = bacc.Bacc("TRN2", target_bir_lowering=False, debug=False,
                   num_devices=N_CORES)

    xT = nc.dram_tensor("xT", [D, S], F32, kind="ExternalInput").ap()
    xTq = nc.dram_tensor("xTq", [D, QH], F32, kind="ExternalInput").ap()
    wq = nc.dram_tensor("wq", [D, H * DK], F32, kind="ExternalInput").ap()
    wk = nc.dram_tensor("wk", [D, H * DK], F32, kind="ExternalInput").ap()
    wv = nc.dram_tensor("wv", [D, H * DV], F32, kind="ExternalInput").ap()
    wo = nc.dram_tensor("wo", [H * DK, DO], F32, kind="ExternalInput").ap()
    bq = nc.dram_tensor("bq", [H * DK], F32, kind="ExternalInput").ap()
    bk = nc.dram_tensor("bk", [H * DK], F32, kind="ExternalInput").ap()
    bv = nc.dram_tensor("bv", [H * DV], F32, kind="ExternalInput").ap()
    bo = nc.dram_tensor("bo", [DO], F32, kind="ExternalInput").ap()
    maskf = nc.dram_tensor("maskf", [S], F32, kind="ExternalInput").ap()
    out = nc.dram_tensor("out", [QH, D], F32, kind="ExternalOutput").ap()

    # DRAM spill for projections (written pass 1, streamed back pass 2/C).
    # Per-head tensors so pass-2 prefetch of head h doesn't wait on head h+1
    # writes (dependency tracking is per-tensor).
    QT_ds = [nc.dram_tensor(f"QT_d{h}", [DK, QH], F32).ap().bitcast(F32R) for h in range(H)]
    KT_ds = [nc.dram_tensor(f"KT_d{h}", [DK, S], F32).ap().bitcast(F32R) for h in range(H)]
    V_ds = [nc.dram_tensor(f"V_d{h}", [S, DV], F32).ap().bitcast(F32R) for h in range(H)]
    G_ds = [nc.dram_tensor(f"G_d{h}", [DK, QH], F32).ap().bitcast(F32R) for h in range(H)]

    with tile.TileContext(nc) as tc:
        with tc.tile_pool(name="consts", bufs=1) as consts:
            # Per-c-tile bias columns (f32: used as tensor_scalar operands).
            bq_cols = consts.tile([128, H * DK // 128], F32)
            bk_cols = consts.tile([128, H * DK // 128], F32)
            for ct in range(H * DK // 128):
                nc.sync.dma_start(out=bq_cols[:, ct:ct + 1],
                                  in_=bq[ct * 128:(ct + 1) * 128].rearrange("(p o) -> p o", o=1))
                nc.sync.dma_start(out=bk_cols[:, ct:ct + 1],
                                  in_=bk[ct * 128:(ct + 1) * 128].rearrange("(p o) -> p o", o=1))
            # Mask columns per k-tile: f32 (scalar operand), x0.5 variant, f32r (matmul lhsT).
            mcol = consts.tile([128, S // 128], F32)
            for kt in range(S // 128):
                nc.sync.dma_start(out=mcol[:, kt:kt + 1],
                                  in_=maskf[kt * 128:(kt + 1) * 128].rearrange("(p o) -> p o", o=1))
            mhalf = consts.tile([128, S // 128], F32)
            nc.vector.tensor_scalar_mul(mhalf, mcol, 0.5)
            mcol_r = consts.tile([128, S // 128], F32R)
            nc.vector.tensor_copy(mcol_r, mcol)
            # Row of ones (bc matmul lhsT, K=1).
            ones_f = consts.tile([1, 128], F32)
            nc.vector.memset(ones_f, 1.0)
            ones1 = consts.tile([1, 128], F32R)
            nc.vector.tensor_copy(ones1, ones_f)
            ones_c = consts.tile([128, 1], F32)
            nc.vector.memset(ones_c, 1.0)
            ones128 = consts.tile([128, 1], F32R)
            nc.vector.tensor_copy(ones128, ones_c)
            # b_o broadcast [128, DO]; plus 0.5*b_o[:D] for the final GLU.
            bo_bc = consts.tile([128, DO], F32)
            nc.gpsimd.dma_start(out=bo_bc, in_=_bcast_ap(bo, 128, 0, DO))
            bo1h = consts.tile([128, D], F32)
            nc.vector.tensor_scalar_mul(bo1h, bo_bc[:, 0:D], 0.5)

            # ---------------- Pass 1: QKV projections -> DRAM ----------------
            with tc.tile_pool(name="p1", bufs=2) as p1, \
                 tc.tile_pool(name="ps1", bufs=6, space="PSUM") as ps1:
                # column-chunked so the first psum group only waits on 2MB
                xT_ch = {}
                xTq_ch = {}
                for kc in range(S // 512):
                    for d in range(D // 128):
                        t = p1.tile([128, 512], F32R, tag="xT_sb", bufs=32, name="xT_c")
                        nc.sync.dma_start(out=t, in_=xT[d * 128:(d + 1) * 128,
                                                       kc * 512:(kc + 1) * 512].bitcast(F32R))
                        xT_ch[(d, kc)] = t
                        if kc < QH // 512:
                            tq2_ = p1.tile([128, 512], F32R, tag="xTq_sb", bufs=16, name="xTq_c")
                            nc.sync.dma_start(out=tq2_, in_=xTq[d * 128:(d + 1) * 128,
                                                              kc * 512:(kc + 1) * 512].bitcast(F32R))
                            xTq_ch[(d, kc)] = tq2_

                for h in range(H):
                    # K^T and Q^T: [dk, seq] via lhsT=W-block [d,128c], rhs=xT [d, seq-chunk]
                    for ct in range(2):
                        c0 = h * DK + ct * 128
                        wkb = []
                        wqb = []
                        for d in range(D // 128):
                            tk = p1.tile([128, 128], F32R, tag="wkb", bufs=24)
                            nc.sync.dma_start(out=tk, in_=wk[d * 128:(d + 1) * 128, c0:c0 + 128].bitcast(F32R))
                            wkb.append(tk)
                            tq = p1.tile([128, 128], F32R, tag="wqb", bufs=24)
                            nc.sync.dma_start(out=tq, in_=wq[d * 128:(d + 1) * 128, c0:c0 + 128].bitcast(F32R))
                            wqb.append(tq)
                        for kc in range(S // 512):
                            ps = ps1.tile([128, 512], F32, tag="ps1")
                            for d in range(D // 128):
                                nc.tensor.matmul(ps, wkb[d], xT_ch[(d, kc)],
                                                 start=(d == 0), stop=(d == D // 128 - 1))
                            ev = p1.tile([128, 512], F32R, tag="ev", bufs=8)
                            nc.vector.tensor_scalar(ev, ps, bk_cols[:, h * 2 + ct:h * 2 + ct + 1],
                                                    None, op0=AL.add)
                            nc.sync.dma_start(out=KT_ds[h][ct * 128:ct * 128 + 128, kc * 512:(kc + 1) * 512], in_=ev)
                        for qc in range(QH // 512):
                            ps = ps1.tile([128, 512], F32, tag="ps1")
                            for d in range(D // 128):
                                nc.tensor.matmul(ps, wqb[d], xTq_ch[(d, qc)],
                                                 start=(d == 0), stop=(d == D // 128 - 1))
                            ev = p1.tile([128, 512], F32R, tag="ev", bufs=8)
                            nc.vector.tensor_scalar(ev, ps, bq_cols[:, h * 2 + ct:h * 2 + ct + 1],
                                                    0.0625, op0=AL.add, op1=AL.mult)
                            nc.sync.dma_start(out=QT_ds[h][ct * 128:ct * 128 + 128, qc * 512:(qc + 1) * 512], in_=ev)
                    # V natural [k, dv]: lhsT=xT-block [d, 128k], rhs=wv [d, 512dv]
                    v0 = h * DV
                    wvb = []
                    for d in range(D // 128):
                        t = p1.tile([128, DV], F32R, tag="wvb", bufs=16)
                        nc.sync.dma_start(out=t, in_=wv[d * 128:(d + 1) * 128, v0:v0 + DV].bitcast(F32R))
                        wvb.append(t)
                    bva = p1.tile([128, 256], F32, tag="bva", bufs=4)
                    nc.gpsimd.dma_start(out=bva, in_=_bcast_ap(bv, 128, v0, 256))
                    bvah = p1.tile([128, 256], F32, tag="bvah", bufs=4)
                    nc.vector.tensor_scalar_mul(bvah, bva, 0.5)
                    bvg = p1.tile([128, 256], F32, tag="bvg", bufs=4)
                    nc.gpsimd.dma_start(out=bvg, in_=_bcast_ap(bv, 128, v0 + 256, 256))
                    for kt in range(S // 128):
                        ps = ps1.tile([128, 512], F32, tag="ps1")
                        for d in range(D // 128):
                            nc.tensor.matmul(ps, xT_ch[(d, kt // 4)][:, (kt % 4) * 128:(kt % 4) * 128 + 128], wvb[d],
                                             start=(d == 0), stop=(d == D // 128 - 1))
                        ev = p1.tile([128, 512], F32R, tag="ev", bufs=8)
                        # a-half gets the extra 0.5 of the GLU-sigmoid identity folded in
                        nc.vector.scalar_tensor_tensor(ev[:, 0:256], ps[:, 0:256],
                                                       mhalf[:, kt:kt + 1], bvah,
                                                       op0=AL.mult, op1=AL.add)
                        nc.vector.scalar_tensor_tensor(ev[:, 256:512], ps[:, 256:512],
                                                       mcol[:, kt:kt + 1], bvg,
                                                       op0=AL.mult, op1=AL.add)
                        nc.sync.dma_start(out=V_ds[h][kt * 128:(kt + 1) * 128, :], in_=ev)

            # ---------------- Pass 2: attention per head ----------------
            with tc.tile_pool(name="p2", bufs=2) as p2, \
                 tc.tile_pool(name="ps_st", bufs=3, space="PSUM") as ps_st, \
                 tc.tile_pool(name="ps_ot", bufs=4, space="PSUM") as ps_ot, \
                 tc.tile_pool(name="ps_dn", bufs=1, space="PSUM") as ps_dn:
                pending_tail = None
                for h in range(H):
                    QT_h = []
                    KT_h = []
                    for ct in range(2):
                        c0 = h * DK + ct * 128
                        tq = p2.tile([128, QH], F32R, tag="qt", bufs=4)
                        nc.sync.dma_start(out=tq, in_=QT_ds[h][ct * 128:ct * 128 + 128, :])
                        QT_h.append(tq)
                        tk = p2.tile([128, S], F32R, tag="kt", bufs=4)
                        nc.sync.dma_start(out=tk, in_=KT_ds[h][ct * 128:ct * 128 + 128, :])
                        KT_h.append(tk)
                    V_h = []
                    for kt in range(S // 128):
                        tv = p2.tile([128, DV], F32R, tag="vt", bufs=24)
                        nc.sync.dma_start(out=tv, in_=V_ds[h][kt * 128:(kt + 1) * 128, :])
                        V_h.append(tv)
                    for qc in range(QH // 512):
                        q0 = qc * 512
                        # previous iteration's tail first: its inputs are ready and it
                        # must lead the DVE/ACT queues so ot psum slots free early.
                        if pending_tail is not None:
                            pending_tail()
                            pending_tail = None
                        ET = []
                        acc = None
                        for kt in range(S // 128):
                            st = ps_st.tile([128, 512], F32, tag="st")
                            nc.tensor.matmul(st, KT_h[0][:, kt * 128:(kt + 1) * 128],
                                             QT_h[0][:, q0:q0 + 512], start=True, stop=False)
                            nc.tensor.matmul(st, KT_h[1][:, kt * 128:(kt + 1) * 128],
                                             QT_h[1][:, q0:q0 + 512], start=False, stop=True)
                            e = p2.tile([128, 512], F32R, tag="et", bufs=28)
                            nc.scalar.activation(e, st, AF.Exp)
                            ET.append(e)
                            # masked-exp running sum on DVE (ping-pong, partition-wise)
                            nacc = p2.tile([128, 512], F32R, tag="acc", bufs=4, name="acc")
                            if acc is None:
                                nc.vector.tensor_scalar(nacc, e, mcol[:, kt:kt + 1],
                                                        None, op0=AL.mult)
                            else:
                                nc.vector.scalar_tensor_tensor(nacc, e, mcol[:, kt:kt + 1],
                                                               acc, op0=AL.mult, op1=AL.add)
                            acc = nacc
                        ots = [ps_ot.tile([128, 512], F32, tag="ot", name=f"ot{_i}") for _i in range(4)]
                        for kt in range(S // 128):
                            for dvt in range(4):
                                nc.tensor.matmul(ots[dvt], V_h[kt][:, dvt * 128:(dvt + 1) * 128],
                                                 ET[kt], start=(kt == 0), stop=(kt == S // 128 - 1))
                        den = ps_dn.tile([1, 512], F32, tag="den")
                        nc.tensor.matmul(den, ones128, acc, start=True, stop=True)
                        dsb = p2.tile([1, 512], F32R, tag="dsb", bufs=2)
                        nc.vector.tensor_copy(dsb, den)
                        bcp = ps_dn.tile([128, 512], F32, tag="den")
                        nc.tensor.matmul(bcp, ones1, dsb, start=True, stop=True)
                        bc = p2.tile([128, 512], F32, tag="bc", bufs=2)
                        nc.vector.reciprocal_approx_fast(bc, bcp)

                        def _tail(h=h, q0=q0, ots=ots, bc=bc):
                            for c2 in range(2):
                                an = p2.tile([128, 512], F32, tag="an", bufs=2, name="an")
                                nc.vector.tensor_tensor(an, ots[c2], bc, AL.mult)
                                gn = p2.tile([128, 512], F32, tag="gn", bufs=2, name="gn")
                                nc.vector.tensor_tensor(gn, ots[2 + c2], bc, AL.mult)
                                tg = p2.tile([128, 512], F32, tag="tg", bufs=2, name="tg")
                                nc.scalar.activation(tg, gn, AF.Tanh, scale=0.5)
                                go = p2.tile([128, 512], F32R, tag="go", bufs=4, name="go")
                                nc.vector.scalar_tensor_tensor(go, tg, 1.0, an,
                                                               op0=AL.add, op1=AL.mult)
                                nc.sync.dma_start(out=G_ds[h][c2 * 128:(c2 + 1) * 128,
                                                              q0:q0 + 512], in_=go)
                        pending_tail = _tail

                if pending_tail is not None:
                    pending_tail()
                    pending_tail = None

            # ---------------- Phase C: output projection + GLU ----------------
            with tc.tile_pool(name="pc", bufs=2) as pc, \
                 tc.tile_pool(name="ps_y", bufs=6, space="PSUM") as ps_y:
                G_sb = []
                for ct in range(H * DK // 128):
                    t = pc.tile([128, QH], F32R, tag="g_sb", bufs=16)
                    nc.sync.dma_start(out=t, in_=G_ds[ct // 2][(ct % 2) * 128:(ct % 2) * 128 + 128, :])
                    G_sb.append(t)
                for pair in range(2):
                    o1c = pair * 512
                    o2c = D + pair * 512
                    wo1 = []
                    wo2 = []
                    for ct in range(H * DK // 128):
                        t1 = pc.tile([128, 512], F32R, tag="wo1", bufs=16)
                        nc.sync.dma_start(out=t1, in_=wo[ct * 128:(ct + 1) * 128,
                                                        o1c:o1c + 512].bitcast(F32R))
                        wo1.append(t1)
                        t2 = pc.tile([128, 512], F32R, tag="wo2", bufs=16)
                        nc.sync.dma_start(out=t2, in_=wo[ct * 128:(ct + 1) * 128,
                                                        o2c:o2c + 512].bitcast(F32R))
                        wo2.append(t2)
                    for qs in range(QH // 128):
                        y1 = ps_y.tile([128, 512], F32, tag="y")
                        y2 = ps_y.tile([128, 512], F32, tag="y")
                        for ct in range(H * DK // 128):
                            nc.tensor.matmul(y1, G_sb[ct][:, qs * 128:(qs + 1) * 128], wo1[ct],
                                             start=(ct == 0), stop=(ct == H * DK // 128 - 1))
                        for ct in range(H * DK // 128):
                            nc.tensor.matmul(y2, G_sb[ct][:, qs * 128:(qs + 1) * 128], wo2[ct],
                                             start=(ct == 0), stop=(ct == H * DK // 128 - 1))
                        y2b = pc.tile([128, 512], F32, tag="y2b", bufs=3)
                        nc.vector.tensor_tensor(y2b, y2, bo_bc[:, o2c:o2c + 512], AL.add)
                        t2 = pc.tile([128, 512], F32, tag="t2", bufs=3)
                        nc.scalar.activation(t2, y2b, AF.Tanh, scale=0.5)
                        y1b = pc.tile([128, 512], F32, tag="y1b", bufs=3)
                        nc.vector.scalar_tensor_tensor(y1b, y1, 0.5, bo1h[:, o1c:o1c + 512],
                                                       op0=AL.mult, op1=AL.add)
                        oc = pc.tile([128, 512], F32, tag="oc", bufs=4)
                        nc.vector.scalar_tensor_tensor(oc, t2, 1.0, y1b,
                                                       op0=AL.add, op1=AL.mult)
                        nc.sync.dma_start(out=out[qs * 128:(qs + 1) * 128, o1c:o1c + 512], in_=oc)

    nc.compile()
    return nc


_NC = None


def kernel(**inputs):
    global _NC
    x = np.ascontiguousarray(np.asarray(inputs["x"], dtype=np.float32))
    mask = np.asarray(inputs["mask"])
    W_q = np.ascontiguousarray(np.asarray(inputs["W_q"], dtype=np.float32))
    W_k = np.ascontiguousarray(np.asarray(inputs["W_k"], dtype=np.float32))
    W_v = np.ascontiguousarray(np.asarray(inputs["W_v"], dtype=np.float32))
    W_o = np.ascontiguousarray(np.asarray(inputs["W_o"], dtype=np.float32))
    b_q = np.ascontiguousarray(np.asarray(inputs["b_q"], dtype=np.float32))
    b_k = np.ascontiguousarray(np.asarray(inputs["b_k"], dtype=np.float32))
    b_v = np.ascontiguousarray(np.asarray(inputs["b_v"], dtype=np.float32))
    b_o = np.ascontiguousarray(np.asarray(inputs["b_o"], dtype=np.float32))

    if _NC is None:
        _NC = _build()

    B = x.shape[0]
    in_maps = []
    for core in range(N_CORES):
        b, g = core // 2, core % 2
        xT = np.ascontiguousarray(x[b].T)
        in_maps.append({
            "xT": xT,
            "xTq": np.ascontiguousarray(xT[:, g * QH:(g + 1) * QH]),
            "wq": W_q, "wk": W_k, "wv": W_v, "wo": W_o,
            "bq": b_q, "bk": b_k, "bv": b_v, "bo": b_o,
            "maskf": np.ascontiguousarray(mask[b].astype(np.float32)),
        })

    res = run_bass_kernel_spmd(_NC, in_maps, core_ids=list(range(N_CORES)))
    out = np.empty((B, S, D), dtype=np.float32)
    for core in range(N_CORES):
        b, g = core // 2, core % 2
        out[b, g * QH:(g + 1) * QH, :] = res.results[core]["out"]
    return out
